# revision 36
# baseline (speedup 1.0000x reference)
"""Single-head attention with QKV projections on 8 TRN2 NeuronCores.

Problem: B=4, S=2048, E=A=1024 f32.
  q = query @ Wq + bq ; k = key @ Wk + bk ; v = value @ Wv + bv
  out = softmax(q k^T / sqrt(A)) v

Sharding: data-parallel over (batch, query-half) -> 8 shards. Both the V and
the K projections are deduplicated across each core pair: every core projects
only its own 1024 rows of K and V, and pair AllGathers (kT in two 1MB halves,
then v) assemble the full tensors in DRAM while later phases keep the PE
busy. The PE streams 128x128x512 bf16 matmuls at the 216ns floor when warm;
with the k-dedup the kernel runs 896 such matmuls (the MAC minimum for this
sharding) plus 16 tiny denominator matmuls.

DMA discipline (the v5 lesson): HWDGE descriptor generation costs ~5.4ns per
contiguous segment on the issuing sequencer, so a 2MB tensor with 2KB rows
costs ~5.5us to issue no matter how the DMAs are batched. All inputs are
therefore supplied by the host in the exact SBUF image layout
([128][et][cols] contiguous, 16KB rows -> 128 segments, ~0.7us), and the
AllGather staging buffers use the same image layout so the gathered kT/v
reload as fully-contiguous member blocks. Staged collective inputs (kst/vst)
ride the Sync queue; bulk input prefetch rides Scalar; the gpsimd queue holds
only the blocking collective_computes.

Phase order: KP (own kT-proj -> agk_in images; AG-k half c fires when its 8
chains land) -> VP (own v-proj + bv fold -> agv_in, AG-v; the fold works
because sum_k probs = 1) -> QP -> S (4 global 512-key chunks streamed from
the gathered images in order 0,2,1,3: scores^T -> exp, Vector accumulates
softmax denominators; no row-max subtraction, |scores| <= ~6) -> AV
(denominator matmuls tucked behind the first AV chain; 1/denom folded into
the PSUM->SBUF output copy). All matmul operands bf16 (PSUM f32); measured
rel_l2 ~5.4e-3.

The v6 lesson (queue discipline for collective consumers): every DMA whose
wait condition is a Collectives>=N semaphore rides the GPSIMD queue,
emitted directly behind its producing collective_compute. Engine queues
execute strictly FIFO, so a collective-gated load on any other queue
head-of-line blocks whatever sits behind it: on Sync it stalled the
producer-gated kst stores (kst-pool WAR -> PSUM backpressure -> 15-25us PE
stalls, plus HAM re-throttle to 1.2GHz after >3.4us idle); on Scalar it
stalled the S-phase EXP activations (10us PE stall via psc-pool WAR). The
gpsimd queue's blocking order IS the dependency order (bias SWDGE loads,
AG-k0, kc00/kc10 loads, AG-k1, kc01/kc11 loads, AG-v, v_sb loads), so its
waits can never delay an instruction that was otherwise ready. wk/xk share
one 4-deep pool so every kc_t slot WAR clears by KP end.
"""
import sys

sys.path.insert(0, "/opt/trn_rl_repo")

import ml_dtypes
import numpy as np

BF16 = ml_dtypes.bfloat16

import concourse.bass as bass
import concourse.tile as tile
from concourse import bacc, bass_utils, mybir

B, S, E, A = 4, 2048, 1024, 1024
SQ = 1024          # queries per core
ET, AT = 8, 8      # 128-tiles of E and A
ST, KT = 16, 16    # 128-tiles of Sk
KCO = 2            # own 512-key chunks (projected locally)
QC, QS, AC = 2, 8, 2    # q 512-chunks, q 128-subtiles, a 512-chunks
SCALE = 1.0 / 32.0      # 1/sqrt(A)
NWARM = 24              # PE warm-up matmuls during the initial DMA window

f32 = mybir.dt.float32
bf16 = mybir.dt.bfloat16
ts = bass.ts


def build():
    nc = bacc.Bacc("TRN2", target_bir_lowering=False, debug=False,
                   dynamic_dma_scratch_size=8192)
    Act = mybir.ActivationFunctionType
    Alu = mybir.AluOpType

    # All bulk inputs are SBUF images: [128, blocks*cols] with 128 contiguous
    # per-partition rows (img[p, blk*cols + j] = src[blk*128 + p, j]).
    xq_d = nc.dram_tensor("xq", [128, ET * SQ], bf16, kind="ExternalInput")
    xk0_d = nc.dram_tensor("xk0", [128, ET * 512], bf16, kind="ExternalInput")
    xk1_d = nc.dram_tensor("xk1", [128, ET * 512], bf16, kind="ExternalInput")
    xv0_d = nc.dram_tensor("xv0", [128, ET * 512], bf16, kind="ExternalInput")
    xv1_d = nc.dram_tensor("xv1", [128, ET * 512], bf16, kind="ExternalInput")
    wq_d = nc.dram_tensor("wq", [128, ET * A], bf16, kind="ExternalInput")
    wk0_d = nc.dram_tensor("wk0", [128, ET * 512], bf16, kind="ExternalInput")
    wk1_d = nc.dram_tensor("wk1", [128, ET * 512], bf16, kind="ExternalInput")
    wv_d = nc.dram_tensor("wv", [128, ET * A], bf16, kind="ExternalInput")
    bqt_d = nc.dram_tensor("bqt", [128, AT], f32, kind="ExternalInput")
    bkt_d = nc.dram_tensor("bkt", [128, AT], f32, kind="ExternalInput")
    bvb_d = nc.dram_tensor("bvb", [128, A], f32, kind="ExternalInput")
    ones_d = nc.dram_tensor("ones", [128, 2], f32, kind="ExternalInput")
    out_d = nc.dram_tensor("out", [SQ, A], f32, kind="ExternalOutput")

    # Long-lived activations as raw (non-pool) SBUF tensors (pool lifetimes
    # are strictly LIFO; these span multiple phase scopes).
    qT = nc.alloc_sbuf_tensor("qT_sb", [128, AT, SQ], bf16).ap()
    v_sb = nc.alloc_sbuf_tensor("v_sb", [128, ST, A], bf16).ap()
    acc = nc.alloc_sbuf_tensor("acc_sb", [128, SQ], f32).ap()
    recip = nc.alloc_sbuf_tensor("recip_sb", [128, QS], f32).ap()
    ones_t = nc.alloc_sbuf_tensor("ones_sb", [128, 2], f32).ap()

    with tile.TileContext(nc) as tc:
        with (
            tc.tile_pool(name="pp512", bufs=2, space="PSUM") as pp512,
            tc.tile_pool(name="pps", bufs=3, space="PSUM") as pps,
            tc.tile_pool(name="pdram", bufs=1, space="DRAM") as pdram,
        ):
            # AllGather staging, image layout: agk_in_c[p, at*512+j] holds
            # own kT for key-chunk c; member blocks concatenate on dim 0.
            agk_in0 = pdram.tile([128, AT * 512], bf16, tag="agki0")
            agk_in1 = pdram.tile([128, AT * 512], bf16, tag="agki1")
            agk_out0 = pdram.tile([256, AT * 512], bf16, tag="agko0")
            agk_out1 = pdram.tile([256, AT * 512], bf16, tag="agko1")
            agv_in = pdram.tile([128, 8 * A], bf16, tag="agvi")
            agv_out = pdram.tile([256, 8 * A], bf16, tag="agvo")
            pe = tc.alloc_tile_pool(name="pe", bufs=1)
            E_t = pe.tile([128, KT, SQ], bf16)  # exp(scores^T) [k, kt, q]
            pwq = tc.alloc_tile_pool(name="pwq", bufs=1)
            pwv = tc.alloc_tile_pool(name="pwv", bufs=1)

            # ---- PE warm-up: dummy matmuls on a zeroed SBUF scrap while the
            # DMA rings boot (~9us) and the first inputs land, so the HAM
            # clock ramp overlaps the dead startup window.
            nc.vector.memset(qT[:, 0, 0:512], 0.0)
            for i in range(NWARM):
                wps = pp512.tile([128, 512], f32, tag="ps", name="wps")
                nc.tensor.matmul(wps[:], qT[:, 0, 0:128], qT[:, 0, 0:512],
                                 start=True, stop=True)

            # ---- KP-phase inputs only; later phases' prefetches are emitted
            # after the KP/VP loops so they do not saturate HBM while the
            # latency-critical agk staging writes stream out. wk and xk share
            # one 4-buffer pool (same 32KB as two 2-buffer pools) so that the
            # four S-phase kc_t loads later reuse slots whose WARs all clear
            # by KP end -- their gens then wait only on the Collectives sems.
            # wk arrives in at-major 256KB blocks (host image is at-major:
            # wk_d[p, (wa et j)] = Wk[et*128+p, (4c+wa)*128+j]) and xk in
            # two 512KB halves, all interleaved in KP consumption order on
            # ONE queue, so KP's first matmul waits for ~768KB instead of
            # 2MB; Sync carries only producer-gated stores.
            pxk = tc.alloc_tile_pool(name="pxk", bufs=4)
            wk_t, xk_t = [None, None], [None, None]
            for c in range(KCO):
                wkd = (wk0_d if c == 0 else wk1_d).ap()
                xkd = (xk0_d if c == 0 else xk1_d).ap()
                wkc = pxk.tile([128, ET, 512], bf16, tag="xk", name="wkc")
                xkc = pxk.tile([128, AT, 512], bf16, tag="xk", name="xkc")
                nc.scalar.dma_start(
                    wkc[:, 0:2, :],
                    wkd[:, 0:1024].rearrange("p (b j) -> p b j", j=512))
                for h in range(2):
                    nc.scalar.dma_start(
                        xkc[:, ts(h, 4), :],
                        xkd[:, h * 2048:(h + 1) * 2048].rearrange(
                            "p (b j) -> p b j", j=512))
                for wa in range(1, 4):
                    nc.scalar.dma_start(
                        wkc[:, ts(wa, 2), :],
                        wkd[:, wa * 1024:(wa + 1) * 1024].rearrange(
                            "p (b j) -> p b j", j=512))
                wk_t[c] = wkc
                xk_t[c] = xkc
            pcs = tc.alloc_tile_pool(name="pcs", bufs=1)
            bkt = pcs.tile([128, AT], f32, tag="bkt")
            nc.gpsimd.dma_start(bkt[:], bkt_d.ap()[:, :])
            bqt = pcs.tile([128, AT], f32, tag="bqt")
            nc.gpsimd.dma_start(bqt[:], bqt_d.ap()[:, :])
            nc.gpsimd.dma_start(ones_t[:], ones_d.ap()[:, :])
            bvb = pcs.tile([128, A], f32, tag="bvb")
            nc.gpsimd.dma_start(bvb[:], bvb_d.ap()[:, :])

            # ---- Phase KP: own kT half = (key_half @ Wk + bk)^T -> agk_in;
            #      one pair AllGather per 512-key column half. The gathered
            #      kc_t chunk loads ride the GPSIMD queue right behind their
            #      producing collective: that queue's blocking order IS the
            #      dependency order, so their Collectives>=N waits can never
            #      head-of-line block anything urgent (on Sync they stalled
            #      the kst/out stores, on Scalar the EXP activations). ----
            pkst = tc.alloc_tile_pool(name="pkst", bufs=12)
            kc_t = {}
            for c in range(KCO):
                agk = agk_in0 if c == 0 else agk_in1
                for at in range(AT):
                    wg, wa = wk_t[at // 4], (at % 4)
                    ps = pp512.tile([128, 512], f32, tag="ps", name="ps_k")
                    for et in range(ET):
                        nc.tensor.matmul(
                            ps[:],
                            wg[:, wa * 2 + et // 4, ts(et % 4, 128)],
                            xk_t[c][:, et, :],
                            start=(et == 0), stop=(et == ET - 1),
                        )
                    kst = pkst.tile([128, 512], bf16, tag="kst", name="kst")
                    nc.vector.tensor_scalar(
                        kst[:], ps[:], bkt[:, at:at + 1], None, Alu.add)
                    nc.sync.dma_start(agk[:, ts(at, 512)], kst[:])
                agko = agk_out0 if c == 0 else agk_out1
                nc.gpsimd.collective_compute(
                    "AllGather",
                    Alu.bypass,
                    ins=[agk.opt()],
                    outs=[agko.opt()],
                    replica_groups=[[0, 1], [2, 3], [4, 5], [6, 7]],
                )
                for m in range(2):
                    kc = pxk.tile([128, AT, 512], bf16, tag="xk",
                                  name="kc_t")
                    nc.gpsimd.dma_start(
                        kc[:],
                        agko[ts(m, 128), :].rearrange(
                            "p (at j) -> p at j", j=512))
                    kc_t[(m, c)] = kc

            # VP-phase prefetch (emitted here so it streams during KP,
            # after the latency-critical KP inputs)
            wv = pwv.tile([128, ET, A], bf16)
            nc.scalar.dma_start(
                wv[:], wv_d.ap().rearrange("p (et j) -> p et j", j=A))
            pxv = tc.alloc_tile_pool(name="pxv", bufs=2)
            xv_c = []
            for sc in range(2):
                xv = pxv.tile([128, ET, 512], bf16, tag="xv", name="xv")
                nc.scalar.dma_start(
                    xv[:],
                    (xv0_d if sc == 0 else xv1_d).ap().rearrange(
                        "p (et j) -> p et j", j=512))
                xv_c.append(xv)

            # ---- Phase VP: own v half = (value_half @ Wv) + bv -> vst
            #      tiles stored straight into the agv_in image, then a pair
            #      AllGather assembles full v in agv_out. The v-bias is
            #      folded in here (sum_k probs = 1 makes it additive), which
            #      drops one vector op from the AV critical path. ----
            pvst = tc.alloc_tile_pool(name="pvst", bufs=2)
            for sc in range(2):          # 512-wide column chunks of the half
                for sti in range(4):
                    stl = sc * 4 + sti   # local s-tile 0..7
                    # one 2-bank PSUM tile; the two 512-col accumulation
                    # chains interleave so each matmul stays within a bank
                    # while the drain is a single 1024-wide vector op
                    ps = pps.tile([128, SQ], f32, tag="psc", name="ps_b")
                    for et in range(ET):
                        for ac in range(AC):
                            nc.tensor.matmul(
                                ps[:, ts(ac, 512)],
                                xv_c[sc][:, et, ts(sti, 128)],
                                wv[:, et, ts(ac, 512)],
                                start=(et == 0), stop=(et == ET - 1),
                            )
                    vst = pvst.tile([128, SQ], bf16, tag="vst", name="vst")
                    nc.vector.tensor_tensor(vst[:], ps[:], bvb[:], Alu.add)
                    nc.sync.dma_start(
                        agv_in[:, stl * A:(stl + 1) * A], vst[:])
            nc.gpsimd.collective_compute(
                "AllGather",
                Alu.bypass,
                ins=[agv_in.opt()],
                outs=[agv_out.opt()],
                replica_groups=[[0, 1], [2, 3], [4, 5], [6, 7]],
            )
            # gathered-v load rides GPSIMD right behind AG-v (see KP note);
            # member m holds global tiles 8m..8m+7
            for m in range(2):
                nc.gpsimd.dma_start(
                    v_sb[:, ts(m, 8), :],
                    agv_out[ts(m, 128), :].rearrange(
                        "p (st j) -> p st j", j=A))

            # QP-phase prefetch (emitted here so it streams during VP).
            # wq arrives in at-major 256KB blocks (host image at-major:
            # wq_d[p, (at et j)] = Wq[et*128+p, at*128+j]) and xq in two
            # halves, ordered by QP consumption so the last-arriving input
            # bytes are also the last-needed (the input stream tail lands
            # ~79us at the shared-HBM rate, after QP begins).
            wq = pwq.tile([128, AT, ET * 128], bf16)
            pxq = tc.alloc_tile_pool(name="pxq", bufs=1)
            xq_t = pxq.tile([128, ET, SQ], bf16)
            wqd = wq_d.ap()
            nc.scalar.dma_start(wq[:, 0, :], wqd[:, 0:1024])
            for h in range(2):
                nc.scalar.dma_start(
                    xq_t[:, ts(h, 4), :],
                    xq_d.ap()[:, h * 4096:(h + 1) * 4096].rearrange(
                        "p (b j) -> p b j", j=SQ))
            for at in range(1, AT):
                nc.scalar.dma_start(
                    wq[:, at, :], wqd[:, at * 1024:(at + 1) * 1024])

            # ---- Phase QP: qT[a, q] = (query @ Wq + bq)^T (after VP so the
            #      v AllGather gets the whole QP+S window to complete) ----
            for at in range(AT):
                ps = pps.tile([128, SQ], f32, tag="psc", name="ps_a")
                for et in range(ET):
                    for qc in range(QC):
                        nc.tensor.matmul(
                            ps[:, ts(qc, 512)], wq[:, at, ts(et, 128)],
                            xq_t[:, et, ts(qc, 512)],
                            start=(et == 0), stop=(et == ET - 1),
                        )
                nc.vector.tensor_scalar(
                    qT[:, at, :], ps[:], bqt[:, at:at + 1],
                    None, Alu.add)

            # ---- Phase S: stream global 512-key chunks from the gathered kT
            #      -> scores^T -> exp; Vector accumulates the denominators.
            #      Chunk order 0,2,1,3: column-half-0 chunks first since the
            #      first AllGather completes ~20us before the second. ----
            nprod = [0]
            first_kt = [None]
            for m, cl in ((0, 0), (1, 0), (0, 1), (1, 1)):
                kc = 2 * m + cl          # global chunk index (key order)
                kc_tile = kc_t[(m, cl)]
                for ki in range(4):
                    kt = kc * 4 + ki
                    psc = pps.tile([128, SQ], f32, tag="psc", name="psc")
                    for at in range(AT):
                        for qc in range(QC):
                            nc.tensor.matmul(
                                psc[:, ts(qc, 512)],
                                kc_tile[:, at, ts(ki, 128)],
                                qT[:, at, ts(qc, 512)],
                                start=(at == 0), stop=(at == AT - 1),
                            )
                    nc.scalar.activation(
                        E_t[:, kt, :], psc[:], Act.Exp,
                        bias=0.0, scale=SCALE)
                    # denominator partial-sums ride along on Vector, in
                    # production order (kt order differs from global order)
                    nprod[0] += 1
                    if nprod[0] == 1:
                        first_kt[0] = kt
                    elif nprod[0] == 2:
                        nc.vector.tensor_tensor(
                            acc[:], E_t[:, first_kt[0], :], E_t[:, kt, :],
                            Alu.add)
                    else:
                        nc.vector.tensor_tensor(
                            acc[:], acc[:], E_t[:, kt, :], Alu.add)

            # ---- Phase AV: out = (probs @ v) * recip; qs-major with one
            #      2-bank PSUM tile per 128-query group (both 512-col
            #      chains interleaved, one 1024-wide drain + store) ----
            pot = tc.alloc_tile_pool(name="pot", bufs=1)
            first_group = [True]
            for qs in range(QS):
                ps = pps.tile([128, SQ], f32, tag="psc", name="ps_av")
                for kt in range(KT):
                    for ac in range(AC):
                        nc.tensor.matmul(
                            ps[:, ts(ac, 512)], E_t[:, kt, ts(qs, 128)],
                            v_sb[:, kt, ts(ac, 512)],
                            start=(kt == 0), stop=(kt == KT - 1),
                        )
                if first_group[0]:
                    # denominators: emitted here so the first AV
                    # group's matmuls cover the acc-chain tail
                    first_group[0] = False
                    for dq in range(QS):
                        psd = pp512.tile([128, 2], f32, tag="ps",
                                         name="psd")
                        nc.tensor.matmul(
                            psd[:], acc[:, ts(dq, 128)], ones_t[:],
                            start=True, stop=True)
                        nc.vector.reciprocal(
                            recip[:, dq:dq + 1], psd[:, 0:1])
                ot = pot.tile([128, SQ], f32, tag="ot", name="ot")
                if qs == QS - 1:
                    # final group drains in halves so the kernel's last
                    # store is 256KB and starts ~1.7us earlier
                    for h in range(2):
                        nc.vector.tensor_scalar(
                            ot[:, ts(h, 512)], ps[:, ts(h, 512)],
                            recip[:, qs:qs + 1], None, Alu.mult)
                        nc.sync.dma_start(
                            out_d.ap()[ts(qs, 128), ts(h, 512)],
                            ot[:, ts(h, 512)])
                else:
                    nc.vector.tensor_scalar(
                        ot[:], ps[:], recip[:, qs:qs + 1], None, Alu.mult)
                    nc.sync.dma_start(out_d.ap()[ts(qs, 128), :], ot[:])

            for p in (pot, pxq, pvst, pxv, pkst, pcs, pxk,
                      pwv, pwq, pe):
                p.release()

    nc.compile()
    return nc


_nc_cache = None


def _get_nc():
    global _nc_cache
    if _nc_cache is None:
        _nc_cache = build()
    return _nc_cache


def _img(xT, c0=None, c1=None):
    """[E, n] -> SBUF image [128, 8*n'] (p-major), optionally column-sliced."""
    t = xT.reshape(ET, 128, xT.shape[1]).transpose(1, 0, 2)
    if c0 is None:
        return np.ascontiguousarray(t.reshape(128, -1))
    return np.ascontiguousarray(t[:, :, c0:c1].reshape(128, -1))


def kernel(query, key, value, Wq, bq, Wk, bk, Wv, bv):
    query = np.asarray(query, dtype=np.float32)
    key = np.asarray(key, dtype=np.float32)
    value = np.asarray(value, dtype=np.float32)
    Wq = np.ascontiguousarray(np.asarray(Wq, dtype=np.float32))
    Wk = np.ascontiguousarray(np.asarray(Wk, dtype=np.float32))
    Wv = np.ascontiguousarray(np.asarray(Wv, dtype=np.float32))
    bq = np.asarray(bq, dtype=np.float32)
    bk = np.asarray(bk, dtype=np.float32)
    bv = np.asarray(bv, dtype=np.float32)

    nc = _get_nc()

    Wq16 = Wq.astype(BF16)
    Wk16 = Wk.astype(BF16)
    Wv16 = Wv.astype(BF16)
    # wq in at-major 256KB blocks (see kernel-side QP loader)
    wq_i = np.concatenate(
        [_img(Wq16, at * 128, (at + 1) * 128) for at in range(AT)], axis=1)
    # wk halves in at-major 256KB blocks (see kernel-side KP loader)
    wk0_i = np.concatenate(
        [_img(Wk16, wa * 128, (wa + 1) * 128) for wa in range(4)], axis=1)
    wk1_i = np.concatenate(
        [_img(Wk16, 512 + wa * 128, 512 + (wa + 1) * 128) for wa in range(4)],
        axis=1)
    wv_i = _img(Wv16)
    bqt = np.ascontiguousarray(bq.reshape(AT, 128).T)
    bkt = np.ascontiguousarray(bk.reshape(AT, 128).T)
    bvb = np.ascontiguousarray(np.broadcast_to(bv, (128, A)))
    ones = np.ones((128, 2), np.float32)

    in_maps = []
    for c in range(8):
        b, h = c // 2, c % 2
        xqT = query[b, h * SQ:(h + 1) * SQ, :].T.astype(BF16)
        xkT = key[b, h * SQ:(h + 1) * SQ, :].T.astype(BF16)
        xvT = value[b, h * SQ:(h + 1) * SQ, :].T.astype(BF16)
        in_maps.append({
            "xq": _img(xqT),
            "xk0": _img(xkT, 0, 512), "xk1": _img(xkT, 512, 1024),
            "xv0": _img(xvT, 0, 512), "xv1": _img(xvT, 512, 1024),
            "wq": wq_i, "wk0": wk0_i, "wk1": wk1_i, "wv": wv_i,
            "bqt": bqt, "bkt": bkt, "bvb": bvb, "ones": ones,
        })

    global _last_in_maps
    _last_in_maps = in_maps
    res = bass_utils.run_bass_kernel_spmd(nc, in_maps, core_ids=list(range(8)))

    out = np.empty((B, S, A), np.float32)
    for c in range(8):
        b, h = c // 2, c % 2
        out[b, h * SQ:(h + 1) * SQ, :] = res.results[c]["out"]
    return out



# revision 37
# speedup vs baseline: 1.2419x; 1.2419x over previous
"""Single-head attention with QKV projections on 8 TRN2 NeuronCores.

Problem: B=4, S=2048, E=A=1024 f32.
  q = query @ Wq + bq ; k = key @ Wk + bk ; v = value @ Wv + bv
  out = softmax(q k^T / sqrt(A)) v

Sharding: data-parallel over (batch, query-half) -> 8 shards. Both the V and
the K projections are deduplicated across each core pair: every core projects
only its own 1024 rows of K and V, and pair AllGathers (kT in two 1MB halves,
then v) assemble the full tensors in DRAM while later phases keep the PE
busy. The PE streams 128x128x512 bf16 matmuls at the 216ns floor when warm;
with the k-dedup the kernel runs 896 such matmuls (the MAC minimum for this
sharding) plus 16 tiny denominator matmuls.

DMA discipline (the v5 lesson): HWDGE descriptor generation costs ~5.4ns per
contiguous segment on the issuing sequencer, so a 2MB tensor with 2KB rows
costs ~5.5us to issue no matter how the DMAs are batched. All inputs are
therefore supplied by the host in the exact SBUF image layout
([128][et][cols] contiguous, 16KB rows -> 128 segments, ~0.7us), and the
AllGather staging buffers use the same image layout so the gathered kT/v
reload as fully-contiguous member blocks. Staged collective inputs (kst/vst)
ride the Sync queue; bulk input prefetch rides Scalar; the gpsimd queue holds
only the blocking collective_computes.

Phase order: KP (own kT-proj -> agk_in images; AG-k half c fires when its 8
chains land) -> VP (own v-proj + bv fold -> agv_in, AG-v; the fold works
because sum_k probs = 1) -> QP -> S (4 global 512-key chunks streamed from
the gathered images in order 0,2,1,3: scores^T -> exp, Vector accumulates
softmax denominators; no row-max subtraction, |scores| <= ~6) -> AV
(denominator matmuls tucked behind the first AV chain; 1/denom folded into
the PSUM->SBUF output copy). All matmul operands bf16 (PSUM f32); measured
rel_l2 ~5.4e-3.

The v6 lesson (queue discipline for collective consumers): every DMA whose
wait condition is a Collectives>=N semaphore rides the GPSIMD queue,
emitted directly behind its producing collective_compute. Engine queues
execute strictly FIFO, so a collective-gated load on any other queue
head-of-line blocks whatever sits behind it: on Sync it stalled the
producer-gated kst stores (kst-pool WAR -> PSUM backpressure -> 15-25us PE
stalls, plus HAM re-throttle to 1.2GHz after >3.4us idle); on Scalar it
stalled the S-phase EXP activations (10us PE stall via psc-pool WAR). The
gpsimd queue's blocking order IS the dependency order (bias SWDGE loads,
AG-k0, kc00/kc10 loads, AG-k1, kc01/kc11 loads, AG-v, v_sb loads), so its
waits can never delay an instruction that was otherwise ready. wk/xk share
one 4-deep pool so every kc_t slot WAR clears by KP end.
"""
import sys

sys.path.insert(0, "/opt/trn_rl_repo")

import ml_dtypes
import numpy as np

BF16 = ml_dtypes.bfloat16

import concourse.bass as bass
import concourse.tile as tile
from concourse import bacc, bass_utils, mybir

B, S, E, A = 4, 2048, 1024, 1024
SQ = 1024          # queries per core
ET, AT = 8, 8      # 128-tiles of E and A
ST, KT = 16, 16    # 128-tiles of Sk
KCO = 2            # own 512-key chunks (projected locally)
QC, QS, AC = 2, 8, 2    # q 512-chunks, q 128-subtiles, a 512-chunks
SCALE = 1.0 / 32.0      # 1/sqrt(A)
NWARM = 24              # PE warm-up matmuls during the initial DMA window

f32 = mybir.dt.float32
bf16 = mybir.dt.bfloat16
ts = bass.ts


def build():
    nc = bacc.Bacc("TRN2", target_bir_lowering=False, debug=False,
                   dynamic_dma_scratch_size=8192)
    Act = mybir.ActivationFunctionType
    Alu = mybir.AluOpType

    # All bulk inputs are SBUF images: [128, blocks*cols] with 128 contiguous
    # per-partition rows (img[p, blk*cols + j] = src[blk*128 + p, j]).
    xq_d = nc.dram_tensor("xq", [128, ET * SQ], bf16, kind="ExternalInput")
    xk0_d = nc.dram_tensor("xk0", [128, ET * 512], bf16, kind="ExternalInput")
    xk1_d = nc.dram_tensor("xk1", [128, ET * 512], bf16, kind="ExternalInput")
    xv0_d = nc.dram_tensor("xv0", [128, ET * 512], bf16, kind="ExternalInput")
    xv1_d = nc.dram_tensor("xv1", [128, ET * 512], bf16, kind="ExternalInput")
    wq_d = nc.dram_tensor("wq", [128, ET * A], bf16, kind="ExternalInput")
    wk0_d = nc.dram_tensor("wk0", [128, ET * 512], bf16, kind="ExternalInput")
    wk1_d = nc.dram_tensor("wk1", [128, ET * 512], bf16, kind="ExternalInput")
    wv_d = nc.dram_tensor("wv", [128, ET * A], bf16, kind="ExternalInput")
    bqt_d = nc.dram_tensor("bqt", [128, AT], f32, kind="ExternalInput")
    bkt_d = nc.dram_tensor("bkt", [128, AT], f32, kind="ExternalInput")
    bvb_d = nc.dram_tensor("bvb", [128, A], f32, kind="ExternalInput")
    ones_d = nc.dram_tensor("ones", [128, 2], f32, kind="ExternalInput")
    out_d = nc.dram_tensor("out", [SQ, A], f32, kind="ExternalOutput")

    # Long-lived activations as raw (non-pool) SBUF tensors (pool lifetimes
    # are strictly LIFO; these span multiple phase scopes).
    qT = nc.alloc_sbuf_tensor("qT_sb", [128, AT, SQ], bf16).ap()
    v_sb = nc.alloc_sbuf_tensor("v_sb", [128, ST, A], bf16).ap()
    acc = nc.alloc_sbuf_tensor("acc_sb", [128, SQ], f32).ap()
    recip = nc.alloc_sbuf_tensor("recip_sb", [128, QS], f32).ap()
    ones_t = nc.alloc_sbuf_tensor("ones_sb", [128, 2], f32).ap()

    with tile.TileContext(nc) as tc:
        with (
            tc.tile_pool(name="pp512", bufs=2, space="PSUM") as pp512,
            tc.tile_pool(name="pps", bufs=3, space="PSUM") as pps,
            tc.tile_pool(name="pdram", bufs=1, space="DRAM") as pdram,
        ):
            # AllGather staging, image layout: agk_in_c[p, at*512+j] holds
            # own kT for key-chunk c; member blocks concatenate on dim 0.
            agk_in0 = pdram.tile([128, AT * 512], bf16, tag="agki0")
            agk_in1 = pdram.tile([128, AT * 512], bf16, tag="agki1")
            agk_out0 = pdram.tile([256, AT * 512], bf16, tag="agko0")
            agk_out1 = pdram.tile([256, AT * 512], bf16, tag="agko1")
            agv_in = pdram.tile([128, 8 * A], bf16, tag="agvi")
            agv_out = pdram.tile([256, 8 * A], bf16, tag="agvo")
            pe = tc.alloc_tile_pool(name="pe", bufs=1)
            E_t = pe.tile([128, KT, SQ], bf16)  # exp(scores^T) [k, kt, q]
            pwq = tc.alloc_tile_pool(name="pwq", bufs=1)
            pwv = tc.alloc_tile_pool(name="pwv", bufs=1)

            # ---- PE warm-up: dummy matmuls on a zeroed SBUF scrap while the
            # DMA rings boot (~9us) and the first inputs land, so the HAM
            # clock ramp overlaps the dead startup window.
            nc.vector.memset(qT[:, 0, 0:512], 0.0)
            for i in range(NWARM):
                wps = pp512.tile([128, 512], f32, tag="ps", name="wps")
                nc.tensor.matmul(wps[:], qT[:, 0, 0:128], qT[:, 0, 0:512],
                                 start=True, stop=True)

            # ---- KP-phase inputs only; later phases' prefetches are emitted
            # after the KP/VP loops so they do not saturate HBM while the
            # latency-critical agk staging writes stream out. wk and xk share
            # one 4-buffer pool (same 32KB as two 2-buffer pools) so that the
            # four S-phase kc_t loads later reuse slots whose WARs all clear
            # by KP end -- their gens then wait only on the Collectives sems.
            # wk arrives in at-major 256KB blocks (host image is at-major:
            # wk_d[p, (wa et j)] = Wk[et*128+p, (4c+wa)*128+j]) and xk in
            # two 512KB halves, all interleaved in KP consumption order on
            # ONE queue, so KP's first matmul waits for ~768KB instead of
            # 2MB; Sync carries only producer-gated stores.
            pxk = tc.alloc_tile_pool(name="pxk", bufs=4)
            wk_t, xk_t = [None, None], [None, None]
            for c in range(KCO):
                wkd = (wk0_d if c == 0 else wk1_d).ap()
                xkd = (xk0_d if c == 0 else xk1_d).ap()
                wkc = pxk.tile([128, ET, 512], bf16, tag="xk", name="wkc")
                xkc = pxk.tile([128, AT, 512], bf16, tag="xk", name="xkc")
                nc.scalar.dma_start(
                    wkc[:, 0:2, :],
                    wkd[:, 0:1024].rearrange("p (b j) -> p b j", j=512))
                for h in range(2):
                    nc.scalar.dma_start(
                        xkc[:, ts(h, 4), :],
                        xkd[:, h * 2048:(h + 1) * 2048].rearrange(
                            "p (b j) -> p b j", j=512))
                for wa in range(1, 4):
                    nc.scalar.dma_start(
                        wkc[:, ts(wa, 2), :],
                        wkd[:, wa * 1024:(wa + 1) * 1024].rearrange(
                            "p (b j) -> p b j", j=512))
                wk_t[c] = wkc
                xk_t[c] = xkc
            pcs = tc.alloc_tile_pool(name="pcs", bufs=1)
            bkt = pcs.tile([128, AT], f32, tag="bkt")
            nc.gpsimd.dma_start(bkt[:], bkt_d.ap()[:, :])
            bqt = pcs.tile([128, AT], f32, tag="bqt")
            nc.gpsimd.dma_start(bqt[:], bqt_d.ap()[:, :])
            nc.gpsimd.dma_start(ones_t[:], ones_d.ap()[:, :])
            bvb = pcs.tile([128, A], f32, tag="bvb")
            nc.gpsimd.dma_start(bvb[:], bvb_d.ap()[:, :])

            # ---- Phase KP: own kT half = (key_half @ Wk + bk)^T -> agk_in;
            #      one pair AllGather per 512-key column half. The gathered
            #      kc_t chunk loads ride the GPSIMD queue right behind their
            #      producing collective: that queue's blocking order IS the
            #      dependency order, so their Collectives>=N waits can never
            #      head-of-line block anything urgent (on Sync they stalled
            #      the kst/out stores, on Scalar the EXP activations). ----
            pkst = tc.alloc_tile_pool(name="pkst", bufs=12)
            kc_t = {}
            for c in range(KCO):
                agk = agk_in0 if c == 0 else agk_in1
                for at in range(AT):
                    wg, wa = wk_t[at // 4], (at % 4)
                    ps = pp512.tile([128, 512], f32, tag="ps", name="ps_k")
                    for et in range(ET):
                        nc.tensor.matmul(
                            ps[:],
                            wg[:, wa * 2 + et // 4, ts(et % 4, 128)],
                            xk_t[c][:, et, :],
                            start=(et == 0), stop=(et == ET - 1),
                        )
                    kst = pkst.tile([128, 512], bf16, tag="kst", name="kst")
                    nc.vector.tensor_scalar(
                        kst[:], ps[:], bkt[:, at:at + 1], None, Alu.add)
                    nc.sync.dma_start(agk[:, ts(at, 512)], kst[:])
                agko = agk_out0 if c == 0 else agk_out1
                nc.gpsimd.collective_compute(
                    "AllGather",
                    Alu.bypass,
                    ins=[agk.opt()],
                    outs=[agko.opt()],
                    replica_groups=[[0, 1], [2, 3], [4, 5], [6, 7]],
                )
                for m in range(2):
                    kc = pxk.tile([128, AT, 512], bf16, tag="xk",
                                  name="kc_t")
                    nc.gpsimd.dma_start(
                        kc[:],
                        agko[ts(m, 128), :].rearrange(
                            "p (at j) -> p at j", j=512))
                    kc_t[(m, c)] = kc

            # VP-phase prefetch (emitted here so it streams during KP,
            # after the latency-critical KP inputs)
            wv = pwv.tile([128, ET, A], bf16)
            nc.scalar.dma_start(
                wv[:], wv_d.ap().rearrange("p (et j) -> p et j", j=A))
            pxv = tc.alloc_tile_pool(name="pxv", bufs=2)
            xv_c = []
            for sc in range(2):
                xv = pxv.tile([128, ET, 512], bf16, tag="xv", name="xv")
                nc.scalar.dma_start(
                    xv[:],
                    (xv0_d if sc == 0 else xv1_d).ap().rearrange(
                        "p (et j) -> p et j", j=512))
                xv_c.append(xv)

            # ---- Phase VP: own v half = (value_half @ Wv) + bv -> vst
            #      tiles stored straight into the agv_in image, then a pair
            #      AllGather assembles full v in agv_out. The v-bias is
            #      folded in here (sum_k probs = 1 makes it additive), which
            #      drops one vector op from the AV critical path. ----
            pvst = tc.alloc_tile_pool(name="pvst", bufs=3)
            for sc in range(2):          # 512-wide column chunks of the half
                for sti in range(4):
                    stl = sc * 4 + sti   # local s-tile 0..7
                    # one 2-bank PSUM tile; the two 512-col accumulation
                    # chains interleave so each matmul stays within a bank
                    # while the drain is a single 1024-wide vector op
                    ps = pps.tile([128, SQ], f32, tag="psc", name="ps_b")
                    for et in range(ET):
                        for ac in range(AC):
                            nc.tensor.matmul(
                                ps[:, ts(ac, 512)],
                                xv_c[sc][:, et, ts(sti, 128)],
                                wv[:, et, ts(ac, 512)],
                                start=(et == 0), stop=(et == ET - 1),
                            )
                    vst = pvst.tile([128, SQ], bf16, tag="vst", name="vst")
                    nc.vector.tensor_tensor(vst[:], ps[:], bvb[:], Alu.add)
                    nc.sync.dma_start(
                        agv_in[:, stl * A:(stl + 1) * A], vst[:])
            nc.gpsimd.collective_compute(
                "AllGather",
                Alu.bypass,
                ins=[agv_in.opt()],
                outs=[agv_out.opt()],
                replica_groups=[[0, 1], [2, 3], [4, 5], [6, 7]],
            )
            # gathered-v load rides GPSIMD right behind AG-v (see KP note);
            # member m holds global tiles 8m..8m+7
            for m in range(2):
                nc.gpsimd.dma_start(
                    v_sb[:, ts(m, 8), :],
                    agv_out[ts(m, 128), :].rearrange(
                        "p (st j) -> p st j", j=A))

            # QP-phase prefetch (emitted here so it streams during VP).
            # wq arrives in at-major 256KB blocks (host image at-major:
            # wq_d[p, (at et j)] = Wq[et*128+p, at*128+j]) and xq in two
            # halves, ordered by QP consumption so the last-arriving input
            # bytes are also the last-needed (the input stream tail lands
            # ~79us at the shared-HBM rate, after QP begins).
            wq = pwq.tile([128, AT, ET * 128], bf16)
            pxq = tc.alloc_tile_pool(name="pxq", bufs=1)
            xq_t = pxq.tile([128, ET, SQ], bf16)
            wqd = wq_d.ap()
            nc.scalar.dma_start(wq[:, 0, :], wqd[:, 0:1024])
            for h in range(2):
                nc.scalar.dma_start(
                    xq_t[:, ts(h, 4), :],
                    xq_d.ap()[:, h * 4096:(h + 1) * 4096].rearrange(
                        "p (b j) -> p b j", j=SQ))
            for at in range(1, AT):
                nc.scalar.dma_start(
                    wq[:, at, :], wqd[:, at * 1024:(at + 1) * 1024])

            # ---- Phase QP: qT[a, q] = (query @ Wq + bq)^T (after VP so the
            #      v AllGather gets the whole QP+S window to complete) ----
            for at in range(AT):
                ps = pps.tile([128, SQ], f32, tag="psc", name="ps_a")
                for et in range(ET):
                    for qc in range(QC):
                        nc.tensor.matmul(
                            ps[:, ts(qc, 512)], wq[:, at, ts(et, 128)],
                            xq_t[:, et, ts(qc, 512)],
                            start=(et == 0), stop=(et == ET - 1),
                        )
                nc.vector.tensor_scalar(
                    qT[:, at, :], ps[:], bqt[:, at:at + 1],
                    None, Alu.add)

            # ---- Phase S: stream global 512-key chunks from the gathered kT
            #      -> scores^T -> exp; Vector accumulates the denominators.
            #      Chunk order 0,2,1,3: column-half-0 chunks first since the
            #      first AllGather completes ~20us before the second. ----
            nprod = [0]
            first_kt = [None]
            for m, cl in ((0, 0), (1, 0), (0, 1), (1, 1)):
                kc = 2 * m + cl          # global chunk index (key order)
                kc_tile = kc_t[(m, cl)]
                for ki in range(4):
                    kt = kc * 4 + ki
                    psc = pps.tile([128, SQ], f32, tag="psc", name="psc")
                    for at in range(AT):
                        for qc in range(QC):
                            nc.tensor.matmul(
                                psc[:, ts(qc, 512)],
                                kc_tile[:, at, ts(ki, 128)],
                                qT[:, at, ts(qc, 512)],
                                start=(at == 0), stop=(at == AT - 1),
                            )
                    nc.scalar.activation(
                        E_t[:, kt, :], psc[:], Act.Exp,
                        bias=0.0, scale=SCALE)
                    # denominator partial-sums ride along on Vector, in
                    # production order (kt order differs from global order)
                    nprod[0] += 1
                    if nprod[0] == 1:
                        first_kt[0] = kt
                    elif nprod[0] == 2:
                        nc.vector.tensor_tensor(
                            acc[:], E_t[:, first_kt[0], :], E_t[:, kt, :],
                            Alu.add)
                    else:
                        nc.vector.tensor_tensor(
                            acc[:], acc[:], E_t[:, kt, :], Alu.add)

            # ---- Phase AV: out = (probs @ v) * recip; qs-major with one
            #      2-bank PSUM tile per 128-query group (both 512-col
            #      chains interleaved, one 1024-wide drain + store) ----
            pot = tc.alloc_tile_pool(name="pot", bufs=1)
            first_group = [True]
            for qs in range(QS):
                ps = pps.tile([128, SQ], f32, tag="psc", name="ps_av")
                for kt in range(KT):
                    for ac in range(AC):
                        nc.tensor.matmul(
                            ps[:, ts(ac, 512)], E_t[:, kt, ts(qs, 128)],
                            v_sb[:, kt, ts(ac, 512)],
                            start=(kt == 0), stop=(kt == KT - 1),
                        )
                if first_group[0]:
                    # denominators: emitted here so the first AV
                    # group's matmuls cover the acc-chain tail
                    first_group[0] = False
                    for dq in range(QS):
                        psd = pp512.tile([128, 2], f32, tag="ps",
                                         name="psd")
                        nc.tensor.matmul(
                            psd[:], acc[:, ts(dq, 128)], ones_t[:],
                            start=True, stop=True)
                        nc.vector.reciprocal(
                            recip[:, dq:dq + 1], psd[:, 0:1])
                ot = pot.tile([128, SQ], f32, tag="ot", name="ot")
                if qs == QS - 1:
                    # final group drains in halves so the kernel's last
                    # store is 256KB and starts ~1.7us earlier
                    for h in range(2):
                        nc.vector.tensor_scalar(
                            ot[:, ts(h, 512)], ps[:, ts(h, 512)],
                            recip[:, qs:qs + 1], None, Alu.mult)
                        nc.sync.dma_start(
                            out_d.ap()[ts(qs, 128), ts(h, 512)],
                            ot[:, ts(h, 512)])
                else:
                    nc.vector.tensor_scalar(
                        ot[:], ps[:], recip[:, qs:qs + 1], None, Alu.mult)
                    nc.sync.dma_start(out_d.ap()[ts(qs, 128), :], ot[:])

            for p in (pot, pxq, pvst, pxv, pkst, pcs, pxk,
                      pwv, pwq, pe):
                p.release()

    nc.compile()
    return nc


_nc_cache = None


def _get_nc():
    global _nc_cache
    if _nc_cache is None:
        _nc_cache = build()
    return _nc_cache


def _img(xT, c0=None, c1=None):
    """[E, n] -> SBUF image [128, 8*n'] (p-major), optionally column-sliced."""
    t = xT.reshape(ET, 128, xT.shape[1]).transpose(1, 0, 2)
    if c0 is None:
        return np.ascontiguousarray(t.reshape(128, -1))
    return np.ascontiguousarray(t[:, :, c0:c1].reshape(128, -1))


def kernel(query, key, value, Wq, bq, Wk, bk, Wv, bv):
    query = np.asarray(query, dtype=np.float32)
    key = np.asarray(key, dtype=np.float32)
    value = np.asarray(value, dtype=np.float32)
    Wq = np.ascontiguousarray(np.asarray(Wq, dtype=np.float32))
    Wk = np.ascontiguousarray(np.asarray(Wk, dtype=np.float32))
    Wv = np.ascontiguousarray(np.asarray(Wv, dtype=np.float32))
    bq = np.asarray(bq, dtype=np.float32)
    bk = np.asarray(bk, dtype=np.float32)
    bv = np.asarray(bv, dtype=np.float32)

    nc = _get_nc()

    Wq16 = Wq.astype(BF16)
    Wk16 = Wk.astype(BF16)
    Wv16 = Wv.astype(BF16)
    # wq in at-major 256KB blocks (see kernel-side QP loader)
    wq_i = np.concatenate(
        [_img(Wq16, at * 128, (at + 1) * 128) for at in range(AT)], axis=1)
    # wk halves in at-major 256KB blocks (see kernel-side KP loader)
    wk0_i = np.concatenate(
        [_img(Wk16, wa * 128, (wa + 1) * 128) for wa in range(4)], axis=1)
    wk1_i = np.concatenate(
        [_img(Wk16, 512 + wa * 128, 512 + (wa + 1) * 128) for wa in range(4)],
        axis=1)
    wv_i = _img(Wv16)
    bqt = np.ascontiguousarray(bq.reshape(AT, 128).T)
    bkt = np.ascontiguousarray(bk.reshape(AT, 128).T)
    bvb = np.ascontiguousarray(np.broadcast_to(bv, (128, A)))
    ones = np.ones((128, 2), np.float32)

    in_maps = []
    for c in range(8):
        b, h = c // 2, c % 2
        xqT = query[b, h * SQ:(h + 1) * SQ, :].T.astype(BF16)
        xkT = key[b, h * SQ:(h + 1) * SQ, :].T.astype(BF16)
        xvT = value[b, h * SQ:(h + 1) * SQ, :].T.astype(BF16)
        in_maps.append({
            "xq": _img(xqT),
            "xk0": _img(xkT, 0, 512), "xk1": _img(xkT, 512, 1024),
            "xv0": _img(xvT, 0, 512), "xv1": _img(xvT, 512, 1024),
            "wq": wq_i, "wk0": wk0_i, "wk1": wk1_i, "wv": wv_i,
            "bqt": bqt, "bkt": bkt, "bvb": bvb, "ones": ones,
        })

    global _last_in_maps
    _last_in_maps = in_maps
    res = bass_utils.run_bass_kernel_spmd(nc, in_maps, core_ids=list(range(8)))

    out = np.empty((B, S, A), np.float32)
    for c in range(8):
        b, h = c // 2, c % 2
        out[b, h * SQ:(h + 1) * SQ, :] = res.results[c]["out"]
    return out



# revision 39
# speedup vs baseline: 1.2483x; 1.0051x over previous
"""Single-head attention with QKV projections on 8 TRN2 NeuronCores.

Problem: B=4, S=2048, E=A=1024 f32.
  q = query @ Wq + bq ; k = key @ Wk + bk ; v = value @ Wv + bv
  out = softmax(q k^T / sqrt(A)) v

Sharding: data-parallel over (batch, query-half) -> 8 shards. Both the V and
the K projections are deduplicated across each core pair: every core projects
only its own 1024 rows of K and V, and pair AllGathers (kT in two 1MB halves,
then v) assemble the full tensors in DRAM while later phases keep the PE
busy. The PE streams 128x128x512 bf16 matmuls at the 216ns floor when warm;
with the k-dedup the kernel runs 896 such matmuls (the MAC minimum for this
sharding) plus 16 tiny denominator matmuls.

DMA discipline (the v5 lesson): HWDGE descriptor generation costs ~5.4ns per
contiguous segment on the issuing sequencer, so a 2MB tensor with 2KB rows
costs ~5.5us to issue no matter how the DMAs are batched. All inputs are
therefore supplied by the host in the exact SBUF image layout
([128][et][cols] contiguous, 16KB rows -> 128 segments, ~0.7us), and the
AllGather staging buffers use the same image layout so the gathered kT/v
reload as fully-contiguous member blocks.

Input streaming (the v7 lesson): the 12MB/core input prefetch is
HBM-bandwidth-bound (~0.17-0.36 MB/us/core with all 8 cores pulling and the
AG meshes sharing HBM mid-window), so the whole stream rides ONE queue
(Scalar) in exact phase-consumption order, with the weights supplied in
at-major 256KB blocks (wk/wq) and activations split in et-halves: each
phase's first matmul then waits for only its first few hundred KB, and the
last-arriving bytes (~80us in) are also the last-needed (late QP blocks).
Sync carries only producer-gated stores (kst/vst/out).

Phase order: KP (own kT-proj -> agk_in images; AG-k half c fires when its 8
chains land) -> VP (own v-proj + bv fold -> agv_in, AG-v; the fold works
because sum_k probs = 1) -> QP -> S (4 global 512-key chunks streamed from
the gathered images in order 0,2,1,3: scores^T -> exp, Vector accumulates
softmax denominators; no row-max subtraction, |scores| <= ~6) -> AV
(denominator matmuls tucked behind the first AV chain; 1/denom folded into
the PSUM->SBUF output copy). All matmul operands bf16 (PSUM f32); measured
rel_l2 ~5.4e-3.

The v6 lesson (queue discipline for collective consumers): every DMA whose
wait condition is a Collectives>=N semaphore rides the GPSIMD queue,
emitted directly behind its producing collective_compute. Engine queues
execute strictly FIFO, so a collective-gated load on any other queue
head-of-line blocks whatever sits behind it: on Sync it stalled the
producer-gated kst stores (kst-pool WAR -> PSUM backpressure -> 15-25us PE
stalls, plus HAM re-throttle to 1.2GHz after >3.4us idle); on Scalar it
stalled the S-phase EXP activations (10us PE stall via psc-pool WAR). The
gpsimd queue's blocking order IS the dependency order (bias SWDGE loads,
AG-k0, kc00/kc10 loads, AG-k1, kc01/kc11 loads, AG-v, v_sb loads), so its
waits can never delay an instruction that was otherwise ready. wk/xk share
one 4-deep pool so every kc_t slot WAR clears by KP end.
"""
import sys

sys.path.insert(0, "/opt/trn_rl_repo")

import ml_dtypes
import numpy as np

BF16 = ml_dtypes.bfloat16

import concourse.bass as bass
import concourse.tile as tile
from concourse import bacc, bass_utils, mybir

B, S, E, A = 4, 2048, 1024, 1024
SQ = 1024          # queries per core
ET, AT = 8, 8      # 128-tiles of E and A
ST, KT = 16, 16    # 128-tiles of Sk
KCO = 2            # own 512-key chunks (projected locally)
QC, QS, AC = 2, 8, 2    # q 512-chunks, q 128-subtiles, a 512-chunks
SCALE = 1.0 / 32.0      # 1/sqrt(A)
NWARM = 24              # PE warm-up matmuls during the initial DMA window

f32 = mybir.dt.float32
bf16 = mybir.dt.bfloat16
ts = bass.ts


def build():
    nc = bacc.Bacc("TRN2", target_bir_lowering=False, debug=False,
                   dynamic_dma_scratch_size=8192)
    Act = mybir.ActivationFunctionType
    Alu = mybir.AluOpType

    # All bulk inputs are SBUF images: [128, blocks*cols] with 128 contiguous
    # per-partition rows (img[p, blk*cols + j] = src[blk*128 + p, j]).
    xq_d = nc.dram_tensor("xq", [128, ET * SQ], bf16, kind="ExternalInput")
    xk0_d = nc.dram_tensor("xk0", [128, ET * 512], bf16, kind="ExternalInput")
    xk1_d = nc.dram_tensor("xk1", [128, ET * 512], bf16, kind="ExternalInput")
    xv0_d = nc.dram_tensor("xv0", [128, ET * 512], bf16, kind="ExternalInput")
    xv1_d = nc.dram_tensor("xv1", [128, ET * 512], bf16, kind="ExternalInput")
    wq_d = nc.dram_tensor("wq", [128, ET * A], bf16, kind="ExternalInput")
    wk0_d = nc.dram_tensor("wk0", [128, ET * 512], bf16, kind="ExternalInput")
    wk1_d = nc.dram_tensor("wk1", [128, ET * 512], bf16, kind="ExternalInput")
    wv_d = nc.dram_tensor("wv", [128, ET * A], bf16, kind="ExternalInput")
    bqt_d = nc.dram_tensor("bqt", [128, AT], f32, kind="ExternalInput")
    bkt_d = nc.dram_tensor("bkt", [128, AT], f32, kind="ExternalInput")
    bvb_d = nc.dram_tensor("bvb", [128, A], f32, kind="ExternalInput")
    ones_d = nc.dram_tensor("ones", [128, 2], f32, kind="ExternalInput")
    out_d = nc.dram_tensor("out", [SQ, A], f32, kind="ExternalOutput")

    # Long-lived activations as raw (non-pool) SBUF tensors (pool lifetimes
    # are strictly LIFO; these span multiple phase scopes).
    qT = nc.alloc_sbuf_tensor("qT_sb", [128, AT, SQ], bf16).ap()
    v_sb = nc.alloc_sbuf_tensor("v_sb", [128, ST, A], bf16).ap()
    acc = nc.alloc_sbuf_tensor("acc_sb", [128, SQ], f32).ap()
    recip = nc.alloc_sbuf_tensor("recip_sb", [128, QS], f32).ap()
    ones_t = nc.alloc_sbuf_tensor("ones_sb", [128, 2], f32).ap()

    with tile.TileContext(nc) as tc:
        with (
            tc.tile_pool(name="pp512", bufs=2, space="PSUM") as pp512,
            tc.tile_pool(name="pps", bufs=3, space="PSUM") as pps,
            tc.tile_pool(name="pdram", bufs=1, space="DRAM") as pdram,
        ):
            # AllGather staging, image layout: agk_in_c[p, at*512+j] holds
            # own kT for key-chunk c; member blocks concatenate on dim 0.
            agk_in0 = pdram.tile([128, AT * 512], bf16, tag="agki0")
            agk_in1 = pdram.tile([128, AT * 512], bf16, tag="agki1")
            agk_out0 = pdram.tile([256, AT * 512], bf16, tag="agko0")
            agk_out1 = pdram.tile([256, AT * 512], bf16, tag="agko1")
            agv_in = pdram.tile([128, 8 * A], bf16, tag="agvi")
            agv_out = pdram.tile([256, 8 * A], bf16, tag="agvo")
            pe = tc.alloc_tile_pool(name="pe", bufs=1)
            E_t = pe.tile([128, KT, SQ], bf16)  # exp(scores^T) [k, kt, q]
            pwq = tc.alloc_tile_pool(name="pwq", bufs=1)
            pwv = tc.alloc_tile_pool(name="pwv", bufs=1)

            # ---- PE warm-up: dummy matmuls on a zeroed SBUF scrap while the
            # DMA rings boot (~9us) and the first inputs land, so the HAM
            # clock ramp overlaps the dead startup window.
            nc.vector.memset(qT[:, 0, 0:512], 0.0)
            for i in range(NWARM):
                wps = pp512.tile([128, 512], f32, tag="ps", name="wps")
                nc.tensor.matmul(wps[:], qT[:, 0, 0:128], qT[:, 0, 0:512],
                                 start=True, stop=True)

            # ---- KP-phase inputs only; later phases' prefetches are emitted
            # after the KP/VP loops so they do not saturate HBM while the
            # latency-critical agk staging writes stream out. wk and xk share
            # one 4-buffer pool (same 32KB as two 2-buffer pools) so that the
            # four S-phase kc_t loads later reuse slots whose WARs all clear
            # by KP end -- their gens then wait only on the Collectives sems.
            # wk arrives in at-major 256KB blocks (host image is at-major:
            # wk_d[p, (wa et j)] = Wk[et*128+p, (4c+wa)*128+j]) and xk in
            # two 512KB halves, all interleaved in KP consumption order on
            # ONE queue, so KP's first matmul waits for ~768KB instead of
            # 2MB; Sync carries only producer-gated stores.
            pxk = tc.alloc_tile_pool(name="pxk", bufs=4)
            wk_t, xk_t = [None, None], [None, None]
            for c in range(KCO):
                wkd = (wk0_d if c == 0 else wk1_d).ap()
                xkd = (xk0_d if c == 0 else xk1_d).ap()
                wkc = pxk.tile([128, ET, 512], bf16, tag="xk", name="wkc")
                xkc = pxk.tile([128, AT, 512], bf16, tag="xk", name="xkc")
                nc.scalar.dma_start(
                    wkc[:, 0:2, :],
                    wkd[:, 0:1024].rearrange("p (b j) -> p b j", j=512))
                for h in range(2):
                    nc.scalar.dma_start(
                        xkc[:, ts(h, 4), :],
                        xkd[:, h * 2048:(h + 1) * 2048].rearrange(
                            "p (b j) -> p b j", j=512))
                for wa in range(1, 4):
                    nc.scalar.dma_start(
                        wkc[:, ts(wa, 2), :],
                        wkd[:, wa * 1024:(wa + 1) * 1024].rearrange(
                            "p (b j) -> p b j", j=512))
                wk_t[c] = wkc
                xk_t[c] = xkc
            pcs = tc.alloc_tile_pool(name="pcs", bufs=1)
            bkt = pcs.tile([128, AT], f32, tag="bkt")
            nc.gpsimd.dma_start(bkt[:], bkt_d.ap()[:, :])
            bqt = pcs.tile([128, AT], f32, tag="bqt")
            nc.gpsimd.dma_start(bqt[:], bqt_d.ap()[:, :])
            nc.gpsimd.dma_start(ones_t[:], ones_d.ap()[:, :])
            bvb = pcs.tile([128, A], f32, tag="bvb")
            nc.gpsimd.dma_start(bvb[:], bvb_d.ap()[:, :])

            # ---- Phase KP: own kT half = (key_half @ Wk + bk)^T -> agk_in;
            #      one pair AllGather per 512-key column half. The gathered
            #      kc_t chunk loads ride the GPSIMD queue right behind their
            #      producing collective: that queue's blocking order IS the
            #      dependency order, so their Collectives>=N waits can never
            #      head-of-line block anything urgent (on Sync they stalled
            #      the kst/out stores, on Scalar the EXP activations). ----
            pkst = tc.alloc_tile_pool(name="pkst", bufs=12)
            kc_t = {}
            for c in range(KCO):
                agk = agk_in0 if c == 0 else agk_in1
                for at in range(AT):
                    wg, wa = wk_t[at // 4], (at % 4)
                    ps = pp512.tile([128, 512], f32, tag="ps", name="ps_k")
                    for et in range(ET):
                        nc.tensor.matmul(
                            ps[:],
                            wg[:, wa * 2 + et // 4, ts(et % 4, 128)],
                            xk_t[c][:, et, :],
                            start=(et == 0), stop=(et == ET - 1),
                        )
                    kst = pkst.tile([128, 512], bf16, tag="kst", name="kst")
                    nc.vector.tensor_scalar(
                        kst[:], ps[:], bkt[:, at:at + 1], None, Alu.add)
                    nc.sync.dma_start(agk[:, ts(at, 512)], kst[:])
                agko = agk_out0 if c == 0 else agk_out1
                nc.gpsimd.collective_compute(
                    "AllGather",
                    Alu.bypass,
                    ins=[agk.opt()],
                    outs=[agko.opt()],
                    replica_groups=[[0, 1], [2, 3], [4, 5], [6, 7]],
                )
                for m in range(2):
                    kc = pxk.tile([128, AT, 512], bf16, tag="xk",
                                  name="kc_t")
                    nc.gpsimd.dma_start(
                        kc[:],
                        agko[ts(m, 128), :].rearrange(
                            "p (at j) -> p at j", j=512))
                    kc_t[(m, c)] = kc

            # VP-phase prefetch (emitted here so it streams during KP,
            # after the latency-critical KP inputs); wv and xv0 interleave
            # in et-halves so VP's first chain waits for ~2MB instead of 3MB
            wv = pwv.tile([128, ET, A], bf16)
            pxv = tc.alloc_tile_pool(name="pxv", bufs=2)
            xv_c = [pxv.tile([128, ET, 512], bf16, tag="xv", name="xv")
                    for _ in range(2)]
            for h in range(2):
                nc.scalar.dma_start(
                    wv[:, ts(h, 4), :],
                    wv_d.ap()[:, h * 4096:(h + 1) * 4096].rearrange(
                        "p (b j) -> p b j", j=A))
                nc.scalar.dma_start(
                    xv_c[0][:, ts(h, 4), :],
                    xv0_d.ap()[:, h * 2048:(h + 1) * 2048].rearrange(
                        "p (b j) -> p b j", j=512))
            for h in range(2):
                nc.scalar.dma_start(
                    xv_c[1][:, ts(h, 4), :],
                    xv1_d.ap()[:, h * 2048:(h + 1) * 2048].rearrange(
                        "p (b j) -> p b j", j=512))

            # ---- Phase VP: own v half = (value_half @ Wv) + bv -> vst
            #      tiles stored straight into the agv_in image, then a pair
            #      AllGather assembles full v in agv_out. The v-bias is
            #      folded in here (sum_k probs = 1 makes it additive), which
            #      drops one vector op from the AV critical path. ----
            pvst = tc.alloc_tile_pool(name="pvst", bufs=3)
            for sc in range(2):          # 512-wide column chunks of the half
                for sti in range(4):
                    stl = sc * 4 + sti   # local s-tile 0..7
                    # one 2-bank PSUM tile; the two 512-col accumulation
                    # chains interleave so each matmul stays within a bank
                    # while the drain is a single 1024-wide vector op
                    ps = pps.tile([128, SQ], f32, tag="psc", name="ps_b")
                    for et in range(ET):
                        for ac in range(AC):
                            nc.tensor.matmul(
                                ps[:, ts(ac, 512)],
                                xv_c[sc][:, et, ts(sti, 128)],
                                wv[:, et, ts(ac, 512)],
                                start=(et == 0), stop=(et == ET - 1),
                            )
                    vst = pvst.tile([128, SQ], bf16, tag="vst", name="vst")
                    nc.vector.tensor_tensor(vst[:], ps[:], bvb[:], Alu.add)
                    nc.sync.dma_start(
                        agv_in[:, stl * A:(stl + 1) * A], vst[:])
            nc.gpsimd.collective_compute(
                "AllGather",
                Alu.bypass,
                ins=[agv_in.opt()],
                outs=[agv_out.opt()],
                replica_groups=[[0, 1], [2, 3], [4, 5], [6, 7]],
            )
            # gathered-v load rides GPSIMD right behind AG-v (see KP note);
            # member m holds global tiles 8m..8m+7
            for m in range(2):
                nc.gpsimd.dma_start(
                    v_sb[:, ts(m, 8), :],
                    agv_out[ts(m, 128), :].rearrange(
                        "p (st j) -> p st j", j=A))

            # QP-phase prefetch (emitted here so it streams during VP).
            # wq arrives in at-major 256KB blocks (host image at-major:
            # wq_d[p, (at et j)] = Wq[et*128+p, at*128+j]) and xq in two
            # halves, ordered by QP consumption so the last-arriving input
            # bytes are also the last-needed (the input stream tail lands
            # ~79us at the shared-HBM rate, after QP begins).
            wq = pwq.tile([128, AT, ET * 128], bf16)
            pxq = tc.alloc_tile_pool(name="pxq", bufs=1)
            xq_t = pxq.tile([128, ET, SQ], bf16)
            wqd = wq_d.ap()
            nc.scalar.dma_start(wq[:, 0, :], wqd[:, 0:1024])
            for h in range(2):
                nc.scalar.dma_start(
                    xq_t[:, ts(h, 4), :],
                    xq_d.ap()[:, h * 4096:(h + 1) * 4096].rearrange(
                        "p (b j) -> p b j", j=SQ))
            for at in range(1, AT):
                nc.scalar.dma_start(
                    wq[:, at, :], wqd[:, at * 1024:(at + 1) * 1024])

            # ---- Phase QP: qT[a, q] = (query @ Wq + bq)^T (after VP so the
            #      v AllGather gets the whole QP+S window to complete) ----
            for at in range(AT):
                ps = pps.tile([128, SQ], f32, tag="psc", name="ps_a")
                for et in range(ET):
                    for qc in range(QC):
                        nc.tensor.matmul(
                            ps[:, ts(qc, 512)], wq[:, at, ts(et, 128)],
                            xq_t[:, et, ts(qc, 512)],
                            start=(et == 0), stop=(et == ET - 1),
                        )
                nc.vector.tensor_scalar(
                    qT[:, at, :], ps[:], bqt[:, at:at + 1],
                    None, Alu.add)

            # ---- Phase S: stream global 512-key chunks from the gathered kT
            #      -> scores^T -> exp; Vector accumulates the denominators.
            #      Chunk order 0,2,1,3: column-half-0 chunks first since the
            #      first AllGather completes ~20us before the second. ----
            nprod = [0]
            first_kt = [None]
            for m, cl in ((0, 0), (1, 0), (0, 1), (1, 1)):
                kc = 2 * m + cl          # global chunk index (key order)
                kc_tile = kc_t[(m, cl)]
                for ki in range(4):
                    kt = kc * 4 + ki
                    psc = pps.tile([128, SQ], f32, tag="psc", name="psc")
                    for at in range(AT):
                        for qc in range(QC):
                            nc.tensor.matmul(
                                psc[:, ts(qc, 512)],
                                kc_tile[:, at, ts(ki, 128)],
                                qT[:, at, ts(qc, 512)],
                                start=(at == 0), stop=(at == AT - 1),
                            )
                    nc.scalar.activation(
                        E_t[:, kt, :], psc[:], Act.Exp,
                        bias=0.0, scale=SCALE)
                    # denominator partial-sums ride along on Vector, in
                    # production order (kt order differs from global order)
                    nprod[0] += 1
                    if nprod[0] == 1:
                        first_kt[0] = kt
                    elif nprod[0] == 2:
                        nc.vector.tensor_tensor(
                            acc[:], E_t[:, first_kt[0], :], E_t[:, kt, :],
                            Alu.add)
                    else:
                        nc.vector.tensor_tensor(
                            acc[:], acc[:], E_t[:, kt, :], Alu.add)

            # ---- Phase AV: out = (probs @ v) * recip; qs-major with one
            #      2-bank PSUM tile per 128-query group (both 512-col
            #      chains interleaved, one 1024-wide drain + store) ----
            pot = tc.alloc_tile_pool(name="pot", bufs=1)
            first_group = [True]
            for qs in range(QS):
                ps = pps.tile([128, SQ], f32, tag="psc", name="ps_av")
                for kt in range(KT):
                    for ac in range(AC):
                        nc.tensor.matmul(
                            ps[:, ts(ac, 512)], E_t[:, kt, ts(qs, 128)],
                            v_sb[:, kt, ts(ac, 512)],
                            start=(kt == 0), stop=(kt == KT - 1),
                        )
                if first_group[0]:
                    # denominators: emitted here so the first AV
                    # group's matmuls cover the acc-chain tail
                    first_group[0] = False
                    for dq in range(QS):
                        psd = pp512.tile([128, 2], f32, tag="ps",
                                         name="psd")
                        nc.tensor.matmul(
                            psd[:], acc[:, ts(dq, 128)], ones_t[:],
                            start=True, stop=True)
                        nc.vector.reciprocal(
                            recip[:, dq:dq + 1], psd[:, 0:1])
                ot = pot.tile([128, SQ], f32, tag="ot", name="ot")
                if qs == QS - 1:
                    # final group drains in halves so the kernel's last
                    # store is 256KB and starts ~1.7us earlier
                    for h in range(2):
                        nc.vector.tensor_scalar(
                            ot[:, ts(h, 512)], ps[:, ts(h, 512)],
                            recip[:, qs:qs + 1], None, Alu.mult)
                        nc.sync.dma_start(
                            out_d.ap()[ts(qs, 128), ts(h, 512)],
                            ot[:, ts(h, 512)])
                else:
                    nc.vector.tensor_scalar(
                        ot[:], ps[:], recip[:, qs:qs + 1], None, Alu.mult)
                    nc.sync.dma_start(out_d.ap()[ts(qs, 128), :], ot[:])

            for p in (pot, pxq, pvst, pxv, pkst, pcs, pxk,
                      pwv, pwq, pe):
                p.release()

    nc.compile()
    return nc


_nc_cache = None


def _get_nc():
    global _nc_cache
    if _nc_cache is None:
        _nc_cache = build()
    return _nc_cache


def _img(xT, c0=None, c1=None):
    """[E, n] -> SBUF image [128, 8*n'] (p-major), optionally column-sliced."""
    t = xT.reshape(ET, 128, xT.shape[1]).transpose(1, 0, 2)
    if c0 is None:
        return np.ascontiguousarray(t.reshape(128, -1))
    return np.ascontiguousarray(t[:, :, c0:c1].reshape(128, -1))


def kernel(query, key, value, Wq, bq, Wk, bk, Wv, bv):
    query = np.asarray(query, dtype=np.float32)
    key = np.asarray(key, dtype=np.float32)
    value = np.asarray(value, dtype=np.float32)
    Wq = np.ascontiguousarray(np.asarray(Wq, dtype=np.float32))
    Wk = np.ascontiguousarray(np.asarray(Wk, dtype=np.float32))
    Wv = np.ascontiguousarray(np.asarray(Wv, dtype=np.float32))
    bq = np.asarray(bq, dtype=np.float32)
    bk = np.asarray(bk, dtype=np.float32)
    bv = np.asarray(bv, dtype=np.float32)

    nc = _get_nc()

    Wq16 = Wq.astype(BF16)
    Wk16 = Wk.astype(BF16)
    Wv16 = Wv.astype(BF16)
    # wq in at-major 256KB blocks (see kernel-side QP loader)
    wq_i = np.concatenate(
        [_img(Wq16, at * 128, (at + 1) * 128) for at in range(AT)], axis=1)
    # wk halves in at-major 256KB blocks (see kernel-side KP loader)
    wk0_i = np.concatenate(
        [_img(Wk16, wa * 128, (wa + 1) * 128) for wa in range(4)], axis=1)
    wk1_i = np.concatenate(
        [_img(Wk16, 512 + wa * 128, 512 + (wa + 1) * 128) for wa in range(4)],
        axis=1)
    wv_i = _img(Wv16)
    bqt = np.ascontiguousarray(bq.reshape(AT, 128).T)
    bkt = np.ascontiguousarray(bk.reshape(AT, 128).T)
    bvb = np.ascontiguousarray(np.broadcast_to(bv, (128, A)))
    ones = np.ones((128, 2), np.float32)

    in_maps = []
    for c in range(8):
        b, h = c // 2, c % 2
        xqT = query[b, h * SQ:(h + 1) * SQ, :].T.astype(BF16)
        xkT = key[b, h * SQ:(h + 1) * SQ, :].T.astype(BF16)
        xvT = value[b, h * SQ:(h + 1) * SQ, :].T.astype(BF16)
        in_maps.append({
            "xq": _img(xqT),
            "xk0": _img(xkT, 0, 512), "xk1": _img(xkT, 512, 1024),
            "xv0": _img(xvT, 0, 512), "xv1": _img(xvT, 512, 1024),
            "wq": wq_i, "wk0": wk0_i, "wk1": wk1_i, "wv": wv_i,
            "bqt": bqt, "bkt": bkt, "bvb": bvb, "ones": ones,
        })

    global _last_in_maps
    _last_in_maps = in_maps
    res = bass_utils.run_bass_kernel_spmd(nc, in_maps, core_ids=list(range(8)))

    out = np.empty((B, S, A), np.float32)
    for c in range(8):
        b, h = c // 2, c % 2
        out[b, h * SQ:(h + 1) * SQ, :] = res.results[c]["out"]
    return out



# revision 40
# speedup vs baseline: 1.3306x; 1.0659x over previous
"""Single-head attention with QKV projections on 8 TRN2 NeuronCores.

Problem: B=4, S=2048, E=A=1024 f32.
  q = query @ Wq + bq ; k = key @ Wk + bk ; v = value @ Wv + bv
  out = softmax(q k^T / sqrt(A)) v

The v8 rewrite (projection folding, collective-free): softmax is invariant
to per-row score offsets, so with W_qk := Wq @ Wk^T (host f32 GEMM, one
bf16 rounding instead of two) and bqk := bq @ Wk^T,
  scores = (query W_qk + bqk) key^T   [+ per-row terms that cancel]
EXACTLY reproduces softmax((query Wq + bq)(key Wk + bk)^T): the dropped
terms (query Wq bk^T and bq bk^T) are constant along each key row. The
V projection re-associates: out = (probs @ value) @ Wv + bv (sum probs = 1
makes bv additive after the @Wv). So the per-core work is
  QP: qT = (query W_qk + bqk)^T            128 matmuls
  S : exp(qT^T key^T / 32) -> E_t          256 matmuls
  AV1: yT = value^T @ E_t                  256 matmuls
  AV2: out = (yT^T @ Wv) * recip + bv      128 matmuls
768 128x128x512-bf16 matmuls/core (vs 896 with on-device K/V projection)
plus 16 tiny denominator matmuls -- and NO collectives: sharding is purely
data-parallel over (batch, query-half); every core receives its batch's
full keyT/value images from the host, so the K/V-projection dedup
AllGathers (and their CC rendezvous + mesh HBM traffic) disappear.

DMA discipline (the v5 lesson): HWDGE descriptor generation costs ~5.4ns
per contiguous segment, so all inputs are host-supplied in the exact SBUF
image layout (contiguous per-partition rows), pre-chunked so every load's
destination is contiguous.

Input streaming (the v7 lesson): the ~14MB/core input prefetch is
HBM-bandwidth-bound (~0.17-0.36 MB/us/core with 8 cores pulling), so the
whole stream rides ONE queue (Scalar) in exact phase-consumption order
(wqk at-major 256KB blocks + xq halves, then keyT in 512-key chunks,
value in 4-kt chunks, wv halves): each phase's first matmul waits only
for its first few hundred KB, and the last-arriving bytes are also the
last-needed. Sync carries only the output stores; gpsimd only the tiny
bias loads. PE warm-up matmuls cover the ~8us DMA-ring boot + first-input
window so the HAM clock gate is at 2.4GHz when QP starts.

All matmul operands bf16 (PSUM f32). No row-max subtraction before exp:
scores ~ N(0,1), |scores| <= ~6. Measured rel_l2 ~5e-3 (tolerance 2e-2).
"""
import sys

sys.path.insert(0, "/opt/trn_rl_repo")

import ml_dtypes
import numpy as np

BF16 = ml_dtypes.bfloat16

import concourse.bass as bass
import concourse.tile as tile
from concourse import bacc, bass_utils, mybir

B, S, E, A = 4, 2048, 1024, 1024
SQ = 1024          # queries per core
ET, AT = 8, 8      # 128-tiles of E and A
KT = 16            # 128-tiles of the 2048 keys
KC = 4             # 512-key chunks
QC, QS, AC = 2, 8, 2    # q 512-chunks, q 128-subtiles, a 512-chunks
SCALE = 1.0 / 32.0      # 1/sqrt(A)
NWARM = 24              # PE warm-up matmuls during the initial DMA window

f32 = mybir.dt.float32
bf16 = mybir.dt.bfloat16
ts = bass.ts


def build():
    nc = bacc.Bacc("TRN2", target_bir_lowering=False, debug=False,
                   dynamic_dma_scratch_size=8192)
    Act = mybir.ActivationFunctionType
    Alu = mybir.AluOpType

    # Host-supplied SBUF images (contiguous per-partition rows):
    #   xq_d  [128, et*1024]: xq[p, et*SQ+j]   = query_half^T[et*128+p, j]
    #   wqk_d [128, at*1024]: at-major blocks, wqk[p, at*1024 + et*128 + j]
    #                         = W_qk[et*128+p, at*128+j]
    #   kt_d  [128, kc*4096]: key-chunk-major, kt_d[p, kc*4096 + et*512 + j]
    #                         = key_b^T[et*128+p, kc*512+j]
    #   xv_d  [128, kt*1024]: value rows tiled by key, xv_d[p, kt*1024+e]
    #                         = value_b[kt*128+p, e]
    #   wv_d  [128, et*1024]: wv_d[p, et*1024+j] = Wv[et*128+p, j]
    xq_d = nc.dram_tensor("xq", [128, ET * SQ], bf16, kind="ExternalInput")
    wqk_d = nc.dram_tensor("wqk", [128, AT * SQ], bf16, kind="ExternalInput")
    kt_d = nc.dram_tensor("ktc", [128, KC * ET * 512], bf16,
                          kind="ExternalInput")
    xv_d = nc.dram_tensor("xv", [128, KT * A], bf16, kind="ExternalInput")
    wv_d = nc.dram_tensor("wv", [128, ET * A], bf16, kind="ExternalInput")
    bqkt_d = nc.dram_tensor("bqkt", [128, AT], f32, kind="ExternalInput")
    bvb_d = nc.dram_tensor("bvb", [128, A], f32, kind="ExternalInput")
    ones_d = nc.dram_tensor("ones", [128, 2], f32, kind="ExternalInput")
    out_d = nc.dram_tensor("out", [SQ, A], f32, kind="ExternalOutput")

    # Long-lived activations as raw (non-pool) SBUF tensors.
    qT = nc.alloc_sbuf_tensor("qT_sb", [128, ET, SQ], bf16).ap()
    kT = nc.alloc_sbuf_tensor("kT_sb", [128, KC, ET, 512], bf16).ap()
    v_sb = nc.alloc_sbuf_tensor("v_sb", [128, KT, A], bf16).ap()
    yT = nc.alloc_sbuf_tensor("yT_sb", [128, ET, SQ], bf16).ap()
    acc = nc.alloc_sbuf_tensor("acc_sb", [128, SQ], f32).ap()
    recip = nc.alloc_sbuf_tensor("recip_sb", [128, QS], f32).ap()
    ones_t = nc.alloc_sbuf_tensor("ones_sb", [128, 2], f32).ap()

    with tile.TileContext(nc) as tc:
        with (
            tc.tile_pool(name="pp512", bufs=2, space="PSUM") as pp512,
            tc.tile_pool(name="pps", bufs=3, space="PSUM") as pps,
        ):
            pe = tc.alloc_tile_pool(name="pe", bufs=1)
            E_t = pe.tile([128, KT, SQ], bf16)  # exp(scores^T) [k, kt, q]
            pwq = tc.alloc_tile_pool(name="pwq", bufs=1)
            pwv = tc.alloc_tile_pool(name="pwv", bufs=1)
            pxq = tc.alloc_tile_pool(name="pxq", bufs=1)
            pcs = tc.alloc_tile_pool(name="pcs", bufs=1)
            pot = tc.alloc_tile_pool(name="pot", bufs=1)

            # ---- PE warm-up: dummy matmuls on a zeroed SBUF scrap while
            # the DMA rings boot (~8us) and the first inputs land, so the
            # HAM clock ramp overlaps the dead startup window.
            nc.vector.memset(qT[:, 0, 0:512], 0.0)
            for i in range(NWARM):
                wps = pp512.tile([128, 512], f32, tag="ps", name="wps")
                nc.tensor.matmul(wps[:], qT[:, 0, 0:128], qT[:, 0, 0:512],
                                 start=True, stop=True)

            # ---- Input stream, ONE queue (Scalar), consumption order ----
            wqk = pwq.tile([128, AT, ET * 128], bf16)
            xq_t = pxq.tile([128, ET, SQ], bf16)
            wqkd = wqk_d.ap()
            nc.scalar.dma_start(wqk[:, 0, :], wqkd[:, 0:1024])
            for h in range(2):
                nc.scalar.dma_start(
                    xq_t[:, ts(h, 4), :],
                    xq_d.ap()[:, h * 4096:(h + 1) * 4096].rearrange(
                        "p (b j) -> p b j", j=SQ))
            for at in range(1, AT):
                nc.scalar.dma_start(
                    wqk[:, at, :], wqkd[:, at * 1024:(at + 1) * 1024])
            for kc in range(KC):     # keyT in S-phase chunk order
                nc.scalar.dma_start(
                    kT[:, kc, :, :],
                    kt_d.ap()[:, kc * 4096:(kc + 1) * 4096].rearrange(
                        "p (et j) -> p et j", j=512))
            for h in range(4):       # value in 4-kt chunks (AV1 order)
                nc.scalar.dma_start(
                    v_sb[:, ts(h, 4), :],
                    xv_d.ap()[:, h * 4096:(h + 1) * 4096].rearrange(
                        "p (kt j) -> p kt j", j=A))
            wv = pwv.tile([128, ET, A], bf16)
            for h in range(2):
                nc.scalar.dma_start(
                    wv[:, ts(h, 4), :],
                    wv_d.ap()[:, h * 4096:(h + 1) * 4096].rearrange(
                        "p (b j) -> p b j", j=A))

            # tiny bias constants ride gpsimd (SWDGE), off both big queues
            bqkt = pcs.tile([128, AT], f32, tag="bqkt")
            nc.gpsimd.dma_start(bqkt[:], bqkt_d.ap()[:, :])
            nc.gpsimd.dma_start(ones_t[:], ones_d.ap()[:, :])
            bvb = pcs.tile([128, A], f32, tag="bvb")
            nc.gpsimd.dma_start(bvb[:], bvb_d.ap()[:, :])

            # ---- Phase QP: qT[e', q] = (query @ W_qk + bqk)^T ----
            for at in range(AT):
                ps = pps.tile([128, SQ], f32, tag="psc", name="ps_a")
                for et in range(ET):
                    for qc in range(QC):
                        nc.tensor.matmul(
                            ps[:, ts(qc, 512)], wqk[:, at, ts(et, 128)],
                            xq_t[:, et, ts(qc, 512)],
                            start=(et == 0), stop=(et == ET - 1),
                        )
                nc.vector.tensor_scalar(
                    qT[:, at, :], ps[:], bqkt[:, at:at + 1], None, Alu.add)

            # ---- Phase S: scores^T = kT-tile @ qT -> exp -> E_t; Vector
            #      accumulates softmax denominators in production order ----
            nprod = 0
            for kc in range(KC):
                for ki in range(4):
                    kt = kc * 4 + ki
                    psc = pps.tile([128, SQ], f32, tag="psc", name="psc")
                    for et in range(ET):
                        for qc in range(QC):
                            nc.tensor.matmul(
                                psc[:, ts(qc, 512)],
                                kT[:, kc, et, ts(ki, 128)],
                                qT[:, et, ts(qc, 512)],
                                start=(et == 0), stop=(et == ET - 1),
                            )
                    nc.scalar.activation(
                        E_t[:, kt, :], psc[:], Act.Exp, bias=0.0,
                        scale=SCALE)
                    nprod += 1
                    if nprod == 2:
                        nc.vector.tensor_tensor(
                            acc[:], E_t[:, 0, :], E_t[:, 1, :], Alu.add)
                    elif nprod > 2:
                        nc.vector.tensor_tensor(
                            acc[:], acc[:], E_t[:, kt, :], Alu.add)

            # ---- Phase AV1: yT[e, q] = value^T @ E_t (unnormalized) ----
            for es in range(ET):
                ps = pps.tile([128, SQ], f32, tag="psc", name="ps_y")
                for kt in range(KT):
                    for qc in range(QC):
                        nc.tensor.matmul(
                            ps[:, ts(qc, 512)], v_sb[:, kt, ts(es, 128)],
                            E_t[:, kt, ts(qc, 512)],
                            start=(kt == 0), stop=(kt == KT - 1),
                        )
                nc.vector.tensor_copy(yT[:, es, :], ps[:])

            # denominators: 128-way partition reduction of acc via tiny
            # matmuls with a ones column; tucked between AV1 and AV2 so
            # the PE covers the last acc adds / yT drain
            for dq in range(QS):
                psd = pp512.tile([128, 2], f32, tag="ps", name="psd")
                nc.tensor.matmul(
                    psd[:], acc[:, ts(dq, 128)], ones_t[:],
                    start=True, stop=True)
                nc.vector.reciprocal(recip[:, dq:dq + 1], psd[:, 0:1])

            # ---- Phase AV2: out = (yT^T @ Wv) * recip + bv ----
            for qs in range(QS):
                ps = pps.tile([128, SQ], f32, tag="psc", name="ps_av")
                for et in range(ET):
                    for ac in range(AC):
                        nc.tensor.matmul(
                            ps[:, ts(ac, 512)], yT[:, et, ts(qs, 128)],
                            wv[:, et, ts(ac, 512)],
                            start=(et == 0), stop=(et == ET - 1),
                        )
                ot = pot.tile([128, SQ], f32, tag="ot", name="ot")
                nhalf = 2 if qs == QS - 1 else 1
                step = SQ // nhalf
                for h in range(nhalf):
                    sl = slice(h * step, (h + 1) * step)
                    nc.vector.tensor_scalar(
                        ot[:, sl], ps[:, sl], recip[:, qs:qs + 1],
                        None, Alu.mult)
                    nc.vector.tensor_tensor(
                        ot[:, sl], ot[:, sl], bvb[:, sl], Alu.add)
                    nc.sync.dma_start(
                        out_d.ap()[ts(qs, 128), sl], ot[:, sl])

            for p in (pot, pcs, pxq, pwv, pwq, pe):
                p.release()

    nc.compile()
    return nc


_nc_cache = None


def _get_nc():
    global _nc_cache
    if _nc_cache is None:
        _nc_cache = build()
    return _nc_cache


def _img(xT, c0=None, c1=None):
    """[E, n] -> SBUF image [128, 8*n'] (p-major), optionally col-sliced."""
    t = xT.reshape(ET, 128, xT.shape[1]).transpose(1, 0, 2)
    if c0 is None:
        return np.ascontiguousarray(t.reshape(128, -1))
    return np.ascontiguousarray(t[:, :, c0:c1].reshape(128, -1))


def kernel(query, key, value, Wq, bq, Wk, bk, Wv, bv):
    query = np.asarray(query, dtype=np.float32)
    key = np.asarray(key, dtype=np.float32)
    value = np.asarray(value, dtype=np.float32)
    Wq = np.ascontiguousarray(np.asarray(Wq, dtype=np.float32))
    Wk = np.ascontiguousarray(np.asarray(Wk, dtype=np.float32))
    Wv = np.ascontiguousarray(np.asarray(Wv, dtype=np.float32))
    bq = np.asarray(bq, dtype=np.float32)
    bk = np.asarray(bk, dtype=np.float32)
    bv = np.asarray(bv, dtype=np.float32)

    nc = _get_nc()

    # Projection folding (see module docstring): scores row-offsets from
    # bk cancel in softmax, so only W_qk and bqk are needed.
    Wqk16 = (Wq @ Wk.T).astype(BF16)
    bqk = bq @ Wk.T                       # [E]
    Wv16 = Wv.astype(BF16)

    wqk_i = np.concatenate(
        [_img(Wqk16, at * 128, (at + 1) * 128) for at in range(AT)], axis=1)
    wv_i = _img(Wv16)
    bqkt = np.ascontiguousarray(bqk.reshape(AT, 128).T)
    bvb = np.ascontiguousarray(np.broadcast_to(bv, (128, A)))
    ones = np.ones((128, 2), np.float32)

    in_maps = []
    for c in range(8):
        b, h = c // 2, c % 2
        xqT = query[b, h * SQ:(h + 1) * SQ, :].T.astype(BF16)
        keyT = key[b].T.astype(BF16)              # [E, 2048]
        val16 = value[b].astype(BF16)             # [2048, E]
        kt_img = np.concatenate(
            [_img(keyT, kc * 512, (kc + 1) * 512) for kc in range(KC)],
            axis=1)
        xv_img = np.ascontiguousarray(
            val16.reshape(KT, 128, A).transpose(1, 0, 2).reshape(128, -1))
        in_maps.append({
            "xq": _img(xqT),
            "wqk": wqk_i,
            "ktc": kt_img,
            "xv": xv_img,
            "wv": wv_i,
            "bqkt": bqkt,
            "bvb": bvb,
            "ones": ones,
        })

    global _last_in_maps
    _last_in_maps = in_maps
    res = bass_utils.run_bass_kernel_spmd(nc, in_maps, core_ids=list(range(8)))

    out = np.empty((B, S, A), np.float32)
    for c in range(8):
        b, h = c // 2, c % 2
        out[b, h * SQ:(h + 1) * SQ, :] = res.results[c]["out"]
    return out


# revision 41
# speedup vs baseline: 1.4543x; 1.0930x over previous
"""Single-head attention with QKV projections on 8 TRN2 NeuronCores.

Problem: B=4, S=2048, E=A=1024 f32.
  q = query @ Wq + bq ; k = key @ Wk + bk ; v = value @ Wv + bv
  out = softmax(q k^T / sqrt(A)) v

The v8 rewrite (projection folding, collective-free): softmax is invariant
to per-row score offsets, so with W_qk := Wq @ Wk^T (host f32 GEMM, one
bf16 rounding instead of two) and bqk := bq @ Wk^T,
  scores = (query W_qk + bqk) key^T   [+ per-row terms that cancel]
EXACTLY reproduces softmax((query Wq + bq)(key Wk + bk)^T): the dropped
terms (query Wq bk^T and bq bk^T) are constant along each key row. The
V projection re-associates: out = (probs @ value) @ Wv + bv (sum probs = 1
makes bv additive after the @Wv). So the per-core work is
  QP: qT = (query W_qk + bqk)^T            128 matmuls
  S : exp(qT^T key^T / 32) -> E_t          256 matmuls
  AV1: yT = value^T @ E_t                  256 matmuls
  AV2: out = (yT^T @ Wv) * recip + bv      128 matmuls
768 128x128x512-bf16 matmuls/core (vs 896 with on-device K/V projection)
plus 16 tiny denominator matmuls -- and NO collectives: sharding is purely
data-parallel over (batch, query-half); every core receives its batch's
full keyT/value images from the host, so the K/V-projection dedup
AllGathers (and their CC rendezvous + mesh HBM traffic) disappear.

DMA discipline (the v5 lesson): HWDGE descriptor generation costs ~5.4ns
per contiguous segment, so all inputs are host-supplied in the exact SBUF
image layout (contiguous per-partition rows), pre-chunked so every load's
destination is contiguous.

Input streaming (the v7 lesson): the ~14MB/core input prefetch is
HBM-bandwidth-bound (~0.17-0.36 MB/us/core with 8 cores pulling), so the
whole stream rides ONE queue (Scalar) in exact phase-consumption order
(wqk at-major 256KB blocks + xq halves, then keyT in 512-key chunks,
value in 4-kt chunks, wv halves): each phase's first matmul waits only
for its first few hundred KB, and the last-arriving bytes are also the
last-needed. Sync carries only the output stores; gpsimd only the tiny
bias loads. PE warm-up matmuls cover the ~8us DMA-ring boot + first-input
window so the HAM clock gate is at 2.4GHz when QP starts.

All matmul operands bf16 (PSUM f32). No row-max subtraction before exp:
scores ~ N(0,1), |scores| <= ~6. Measured rel_l2 ~5e-3 (tolerance 2e-2).
"""
import sys

sys.path.insert(0, "/opt/trn_rl_repo")

import ml_dtypes
import numpy as np

BF16 = ml_dtypes.bfloat16

import concourse.bass as bass
import concourse.tile as tile
from concourse import bacc, bass_utils, mybir

B, S, E, A = 4, 2048, 1024, 1024
SQ = 1024          # queries per core
ET, AT = 8, 8      # 128-tiles of E and A
KT = 16            # 128-tiles of the 2048 keys
KC = 4             # 512-key chunks
QC, QS, AC = 2, 8, 2    # q 512-chunks, q 128-subtiles, a 512-chunks
SCALE = 1.0 / 32.0      # 1/sqrt(A)
NWARM = 24              # PE warm-up matmuls during the initial DMA window

f32 = mybir.dt.float32
bf16 = mybir.dt.bfloat16
ts = bass.ts


def build():
    nc = bacc.Bacc("TRN2", target_bir_lowering=False, debug=False,
                   dynamic_dma_scratch_size=8192)
    Act = mybir.ActivationFunctionType
    Alu = mybir.AluOpType

    # Host-supplied SBUF images (contiguous per-partition rows):
    #   xq_d  [128, et*1024]: xq[p, et*SQ+j]   = query_half^T[et*128+p, j]
    #   wqk_d [128, at*1024]: at-major blocks, wqk[p, at*1024 + et*128 + j]
    #                         = W_qk[et*128+p, at*128+j]
    #   kt_d  [128, kc*4096]: key-chunk-major, kt_d[p, kc*4096 + et*512 + j]
    #                         = key_b^T[et*128+p, kc*512+j]
    #   xv_d  [128, kt*1024]: value rows tiled by key, xv_d[p, kt*1024+e]
    #                         = value_b[kt*128+p, e]
    #   wv_d  [128, et*1024]: wv_d[p, et*1024+j] = Wv[et*128+p, j]
    xq_d = nc.dram_tensor("xq", [128, ET * SQ], bf16, kind="ExternalInput")
    wqk_d = nc.dram_tensor("wqk", [128, AT * SQ], bf16, kind="ExternalInput")
    kt_d = nc.dram_tensor("ktc", [128, KC * ET * 512], bf16,
                          kind="ExternalInput")
    xv_d = nc.dram_tensor("xv", [128, KT * A], bf16, kind="ExternalInput")
    wv_d = nc.dram_tensor("wv", [128, ET * A], bf16, kind="ExternalInput")
    bqkt_d = nc.dram_tensor("bqkt", [128, AT], f32, kind="ExternalInput")
    bvb_d = nc.dram_tensor("bvb", [128, A], f32, kind="ExternalInput")
    ones_d = nc.dram_tensor("ones", [128, 2], f32, kind="ExternalInput")
    out_d = nc.dram_tensor("out", [SQ, A], f32, kind="ExternalOutput")

    # Long-lived activations as raw (non-pool) SBUF tensors.
    qT = nc.alloc_sbuf_tensor("qT_sb", [128, ET, SQ], bf16).ap()
    kT = nc.alloc_sbuf_tensor("kT_sb", [128, KC, ET, 512], bf16).ap()
    v_sb = nc.alloc_sbuf_tensor("v_sb", [128, KT, A], bf16).ap()
    yT = nc.alloc_sbuf_tensor("yT_sb", [128, ET, SQ], bf16).ap()
    acc = nc.alloc_sbuf_tensor("acc_sb", [128, SQ], f32).ap()
    recip = nc.alloc_sbuf_tensor("recip_sb", [128, QS], f32).ap()
    ones_t = nc.alloc_sbuf_tensor("ones_sb", [128, 2], f32).ap()

    with tile.TileContext(nc) as tc:
        with (
            tc.tile_pool(name="pp512", bufs=2, space="PSUM") as pp512,
            tc.tile_pool(name="pps", bufs=3, space="PSUM") as pps,
        ):
            pe = tc.alloc_tile_pool(name="pe", bufs=1)
            E_t = pe.tile([128, KT, SQ], bf16)  # exp(scores^T) [k, kt, q]
            pwq = tc.alloc_tile_pool(name="pwq", bufs=1)
            pwv = tc.alloc_tile_pool(name="pwv", bufs=1)
            pxq = tc.alloc_tile_pool(name="pxq", bufs=1)
            pcs = tc.alloc_tile_pool(name="pcs", bufs=1)
            pot = tc.alloc_tile_pool(name="pot", bufs=3)

            # ---- PE warm-up: dummy matmuls on a zeroed SBUF scrap while
            # the DMA rings boot (~8us) and the first inputs land, so the
            # HAM clock ramp overlaps the dead startup window.
            nc.vector.memset(qT[:, 0, 0:512], 0.0)
            for i in range(NWARM):
                wps = pp512.tile([128, 512], f32, tag="ps", name="wps")
                nc.tensor.matmul(wps[:], qT[:, 0, 0:128], qT[:, 0, 0:512],
                                 start=True, stop=True)

            # ---- Input stream, ONE queue (Scalar), consumption order ----
            wqk = pwq.tile([128, AT, ET * 128], bf16)
            xq_t = pxq.tile([128, ET, SQ], bf16)
            wqkd = wqk_d.ap()
            nc.scalar.dma_start(wqk[:, 0, :], wqkd[:, 0:1024])
            for h in range(2):
                nc.scalar.dma_start(
                    xq_t[:, ts(h, 4), :],
                    xq_d.ap()[:, h * 4096:(h + 1) * 4096].rearrange(
                        "p (b j) -> p b j", j=SQ))
            for at in range(1, AT):
                nc.scalar.dma_start(
                    wqk[:, at, :], wqkd[:, at * 1024:(at + 1) * 1024])
            for kc in range(KC):     # keyT in S-phase chunk order
                nc.scalar.dma_start(
                    kT[:, kc, :, :],
                    kt_d.ap()[:, kc * 4096:(kc + 1) * 4096].rearrange(
                        "p (et j) -> p et j", j=512))
            for h in range(4):       # value in 4-kt chunks (AV1 order)
                nc.scalar.dma_start(
                    v_sb[:, ts(h, 4), :],
                    xv_d.ap()[:, h * 4096:(h + 1) * 4096].rearrange(
                        "p (kt j) -> p kt j", j=A))
            wv = pwv.tile([128, ET, A], bf16)
            for h in range(2):
                nc.scalar.dma_start(
                    wv[:, ts(h, 4), :],
                    wv_d.ap()[:, h * 4096:(h + 1) * 4096].rearrange(
                        "p (b j) -> p b j", j=A))

            # tiny bias constants ride gpsimd (SWDGE), off both big queues
            bqkt = pcs.tile([128, AT], f32, tag="bqkt")
            nc.gpsimd.dma_start(bqkt[:], bqkt_d.ap()[:, :])
            nc.gpsimd.dma_start(ones_t[:], ones_d.ap()[:, :])
            bvb = pcs.tile([128, A], f32, tag="bvb")
            nc.gpsimd.dma_start(bvb[:], bvb_d.ap()[:, :])

            # ---- Phase QP: qT[e', q] = (query @ W_qk + bqk)^T ----
            for at in range(AT):
                ps = pps.tile([128, SQ], f32, tag="psc", name="ps_a")
                for et in range(ET):
                    for qc in range(QC):
                        nc.tensor.matmul(
                            ps[:, ts(qc, 512)], wqk[:, at, ts(et, 128)],
                            xq_t[:, et, ts(qc, 512)],
                            start=(et == 0), stop=(et == ET - 1),
                        )
                nc.vector.tensor_scalar(
                    qT[:, at, :], ps[:], bqkt[:, at:at + 1], None, Alu.add)

            # ---- Phase S: scores^T = kT-tile @ qT -> exp -> E_t; Vector
            #      accumulates softmax denominators in production order ----
            nprod = 0
            for kc in range(KC):
                for ki in range(4):
                    kt = kc * 4 + ki
                    psc = pps.tile([128, SQ], f32, tag="psc", name="psc")
                    for et in range(ET):
                        for qc in range(QC):
                            nc.tensor.matmul(
                                psc[:, ts(qc, 512)],
                                kT[:, kc, et, ts(ki, 128)],
                                qT[:, et, ts(qc, 512)],
                                start=(et == 0), stop=(et == ET - 1),
                            )
                    nc.scalar.activation(
                        E_t[:, kt, :], psc[:], Act.Exp, bias=0.0,
                        scale=SCALE)
                    nprod += 1
                    if nprod == 2:
                        nc.vector.tensor_tensor(
                            acc[:], E_t[:, 0, :], E_t[:, 1, :], Alu.add)
                    elif nprod > 2:
                        nc.vector.tensor_tensor(
                            acc[:], acc[:], E_t[:, kt, :], Alu.add)

            # ---- Phase AV1: yT[e, q] = value^T @ E_t (unnormalized) ----
            for es in range(ET):
                ps = pps.tile([128, SQ], f32, tag="psc", name="ps_y")
                for kt in range(KT):
                    for qc in range(QC):
                        nc.tensor.matmul(
                            ps[:, ts(qc, 512)], v_sb[:, kt, ts(es, 128)],
                            E_t[:, kt, ts(qc, 512)],
                            start=(kt == 0), stop=(kt == KT - 1),
                        )
                nc.vector.tensor_copy(yT[:, es, :], ps[:])

            # denominators: 128-way partition reduction of acc via tiny
            # matmuls with a ones column; tucked between AV1 and AV2 so
            # the PE covers the last acc adds / yT drain
            for dq in range(QS):
                psd = pp512.tile([128, 2], f32, tag="ps", name="psd")
                nc.tensor.matmul(
                    psd[:], acc[:, ts(dq, 128)], ones_t[:],
                    start=True, stop=True)
                nc.vector.reciprocal(recip[:, dq:dq + 1], psd[:, 0:1])

            # ---- Phase AV2: out = (yT^T @ Wv) * recip + bv ----
            for qs in range(QS):
                ps = pps.tile([128, SQ], f32, tag="psc", name="ps_av")
                for et in range(ET):
                    for ac in range(AC):
                        nc.tensor.matmul(
                            ps[:, ts(ac, 512)], yT[:, et, ts(qs, 128)],
                            wv[:, et, ts(ac, 512)],
                            start=(et == 0), stop=(et == ET - 1),
                        )
                ot = pot.tile([128, SQ], f32, tag="ot", name="ot")
                nhalf = 2 if qs == QS - 1 else 1
                step = SQ // nhalf
                for h in range(nhalf):
                    sl = slice(h * step, (h + 1) * step)
                    nc.vector.tensor_scalar(
                        ot[:, sl], ps[:, sl], recip[:, qs:qs + 1],
                        None, Alu.mult)
                    nc.vector.tensor_tensor(
                        ot[:, sl], ot[:, sl], bvb[:, sl], Alu.add)
                    nc.sync.dma_start(
                        out_d.ap()[ts(qs, 128), sl], ot[:, sl])

            for p in (pot, pcs, pxq, pwv, pwq, pe):
                p.release()

    nc.compile()
    return nc


_nc_cache = None


def _get_nc():
    global _nc_cache
    if _nc_cache is None:
        _nc_cache = build()
    return _nc_cache


def _img(xT, c0=None, c1=None):
    """[E, n] -> SBUF image [128, 8*n'] (p-major), optionally col-sliced."""
    t = xT.reshape(ET, 128, xT.shape[1]).transpose(1, 0, 2)
    if c0 is None:
        return np.ascontiguousarray(t.reshape(128, -1))
    return np.ascontiguousarray(t[:, :, c0:c1].reshape(128, -1))


def kernel(query, key, value, Wq, bq, Wk, bk, Wv, bv):
    query = np.asarray(query, dtype=np.float32)
    key = np.asarray(key, dtype=np.float32)
    value = np.asarray(value, dtype=np.float32)
    Wq = np.ascontiguousarray(np.asarray(Wq, dtype=np.float32))
    Wk = np.ascontiguousarray(np.asarray(Wk, dtype=np.float32))
    Wv = np.ascontiguousarray(np.asarray(Wv, dtype=np.float32))
    bq = np.asarray(bq, dtype=np.float32)
    bk = np.asarray(bk, dtype=np.float32)
    bv = np.asarray(bv, dtype=np.float32)

    nc = _get_nc()

    # Projection folding (see module docstring): scores row-offsets from
    # bk cancel in softmax, so only W_qk and bqk are needed.
    Wqk16 = (Wq @ Wk.T).astype(BF16)
    bqk = bq @ Wk.T                       # [E]
    Wv16 = Wv.astype(BF16)

    wqk_i = np.concatenate(
        [_img(Wqk16, at * 128, (at + 1) * 128) for at in range(AT)], axis=1)
    wv_i = _img(Wv16)
    bqkt = np.ascontiguousarray(bqk.reshape(AT, 128).T)
    bvb = np.ascontiguousarray(np.broadcast_to(bv, (128, A)))
    ones = np.ones((128, 2), np.float32)

    in_maps = []
    for c in range(8):
        b, h = c // 2, c % 2
        xqT = query[b, h * SQ:(h + 1) * SQ, :].T.astype(BF16)
        keyT = key[b].T.astype(BF16)              # [E, 2048]
        val16 = value[b].astype(BF16)             # [2048, E]
        kt_img = np.concatenate(
            [_img(keyT, kc * 512, (kc + 1) * 512) for kc in range(KC)],
            axis=1)
        xv_img = np.ascontiguousarray(
            val16.reshape(KT, 128, A).transpose(1, 0, 2).reshape(128, -1))
        in_maps.append({
            "xq": _img(xqT),
            "wqk": wqk_i,
            "ktc": kt_img,
            "xv": xv_img,
            "wv": wv_i,
            "bqkt": bqkt,
            "bvb": bvb,
            "ones": ones,
        })

    global _last_in_maps
    _last_in_maps = in_maps
    res = bass_utils.run_bass_kernel_spmd(nc, in_maps, core_ids=list(range(8)))

    out = np.empty((B, S, A), np.float32)
    for c in range(8):
        b, h = c // 2, c % 2
        out[b, h * SQ:(h + 1) * SQ, :] = res.results[c]["out"]
    return out
